# revision 50
# baseline (speedup 1.0000x reference)
"""Trainium2 Bass kernel for gated multi-head pair attention (AlphaFold-style).

Reference computation (B=1, N=256, C=128, H=4, DH=32):
    q = (q_data @ wq.T) * DH**-0.5        # [B,N,Nq,C]
    k = m_data @ wk.T ; v = m_data @ wv.T
    logits = einsum("bnqhd,bnkhd->bnhqk", q, k) + bias + nonbatched_bias
    weight = softmax(logits, axis=-1)
    wa = einsum("bnhqk,bnkhd->bnqhd", weight, v)
    g  = sigmoid(q_data @ wg.T + bg)
    out = (wa * g).reshape(...) @ wo.T + bo

Sharding: pure data-parallel across the 8 NeuronCores along the first
residue axis (N): core c owns rows [32c, 32c+32). Params + nonbatched_bias
replicated.

v4 (four-engine pipeline): k-major like v2/v3, restructured around the
measured per-engine workload (ACT was the binding engine at ~3.45us/row):
  - four-engine split: ACT keeps only exp/ln work (et-exps, gate-exp,
    batched ln+rs).  GPSIMD (idle before) takes the kc0 et*exp(nb)
    multiply and the wag=wa*rs multiply (SBUF-only tensor ops DO compile
    on this build; PSUM access does not).  DVE keeps the PSUM drains, the
    kc1 multiply, d=(1+e1)*S, and the out staging copy.
  - deeper software pipeline: logits+exp for rows (2u-2, 2u-1) and
    waU/S accumulation for rows (2u-4, 2u-3) run in superslot u, so the
    slow GPSIMD multiply (~2.1us) is off the PE's in-order critical path.
  - wag is computed at rs-batch completion time (not at out-pop time) so
    the out matmul never waits on GPSIMD latency.
  - DMA: consts packed into two blobs; input DMAs split across the SP and
    ACT hardware DGE queues (fblob+nbT on ACT, xt/mt chunks on SP) so the
    nonbatched-bias transfer doesn't serialize behind the input chunks;
    first chunk is only 2 rows so row 0 can start ~4us earlier.
  - exp(nb) is loaded bf16 and exp'd once; bias folded into the et-exp's
    per-partition bias port; softmax denom and sigmoid gate fused into
    ONE reciprocal chain rs = exp(-ln((1+e1)*S)).

Environment notes (this walrus build): one sem wait max per instruction
(_legalize_multiwaits); two matmuls must never concurrently target
different column ranges of the same PSUM bank; gpsimd cannot access PSUM;
no PSUM-source DMAs; only exp/ln ACT funcs are used so the ACT table set
loads exactly once.  Measured: the 4 tile-positioned matmuls of a group
stream their output columns serially (cost = total cols at 2.4GHz);
lg-bank kc double-buffering is SLOWER (PSUM port contention with the
exp's reads); gpsimd Multiply eff=0.42 -> [128,1024] mul ~2.1us.
"""

import os
import sys

sys.path.insert(0, "/opt/trn_rl_repo")

from contextlib import ExitStack

import numpy as np

import concourse.bass as bass
import concourse.tile as tile
from concourse import mybir
from concourse.bass_utils import run_bass_kernel_spmd

B, N, C, H = 1, 256, 128, 4
DH = C // H
KEY_SCALE = DH**-0.5
NCORES = 8
RPC = N // NCORES  # rows per core
G = 4  # rows per batched ln/rs epilogue call
CHUNK_ROWS = [2, 6, 8, 8, 8]  # uneven input chunks: row 0 starts early

F32 = mybir.dt.float32
BF16 = mybir.dt.bfloat16

WITH_BO = True  # set by kernel() per-input; bo==0 skips the bias matmuls

_CACHE = {}


def _legalize_multiwaits(nc, max_waits=1):
    """The walrus build here encodes at most one sem wait per instruction
    ("Too many sync wait commands" otherwise). Split excess waits onto
    freshly inserted Drain instructions on the same engine just before the
    multi-wait instruction (engines execute in order, so this is
    equivalent)."""
    n_fix = 0
    for f in nc.m.functions:
        for blk in f.blocks:
            changed = False
            new_insts = []
            for inst in blk.instructions:
                si = inst.sync_info
                ow = list(si.on_wait) if (si is not None and si.on_wait) else []
                if len(ow) > max_waits:
                    head, tail = ow[:-max_waits], ow[-max_waits:]
                    while head:
                        chunk, head = head[:max_waits], head[max_waits:]
                        d = mybir.InstNoOp(
                            name=f"I-mw{nc.next_id()}", ins=[], outs=[]
                        )
                        d.engine = inst.engine
                        d.sync_info = mybir.SyncInfo(
                            on_wait=list(chunk), on_update=[]
                        )
                        new_insts.append(d)
                        n_fix += 1
                    inst.sync_info = mybir.SyncInfo(
                        on_wait=list(tail),
                        on_update=list(si.on_update) if si.on_update else [],
                    )
                    changed = True
                new_insts.append(inst)
            if changed:
                blk.instructions = new_insts
    return n_fix


# row -> (chunk index, row offset inside chunk)
_CH_OF = []
for _ci, _n in enumerate(CHUNK_ROWS):
    for _j in range(_n):
        _CH_OF.append((_ci, _j))
_CH_BASE = [sum(CHUNK_ROWS[:i]) for i in range(len(CHUNK_ROWS))]


def _emit(ctx: ExitStack, tc: "tile.TileContext", t):
    nc = tc.nc
    MM = nc.tensor.matmul
    Act = mybir.ActivationFunctionType
    NB = 512  # psum bank stride (fp32 elems)

    const = ctx.enter_context(tc.tile_pool(name="const", bufs=1))

    # warmup rhs + ones tiles first: memset-only (no DMA dependency) so the
    # PE ramp burst can start immediately.
    warm_sb = const.tile([128, N], BF16, name="warm_sb")
    nc.vector.memset(warm_sb, 0.0)
    ones32_sb = const.tile([128, DH], BF16)
    nc.vector.memset(ones32_sb, 1.0)
    if WITH_BO:
        ones512_sb = const.tile([1, 2 * N], BF16)
        nc.vector.memset(ones512_sb, 1.0)

    # consolidated const DMAs, ordered for the row-0 critical path on the
    # SP queue: wblob -> xt/mt chunk 0 -> remaining chunks.  fblob + nbT
    # go down the ACT engine's independent HW DGE queue in parallel.
    wblob_sb = const.tile([128, 5 * C], BF16, name="wblob_sb")
    nc.sync.dma_start(wblob_sb, t["wblob"].ap())
    wq_sb = wblob_sb[:, 0 * C : 1 * C]
    wk_sb = wblob_sb[:, 1 * C : 2 * C]
    wv_sb = wblob_sb[:, 2 * C : 3 * C]
    wg_sb = wblob_sb[:, 3 * C : 4 * C]
    wo_sb = wblob_sb[:, 4 * C : 5 * C]

    xt_ch = [
        const.tile([128, n * N], BF16, name=f"xt_ch{i}")
        for i, n in enumerate(CHUNK_ROWS)
    ]
    mt_ch = [
        const.tile([128, n * N], BF16, name=f"mt_ch{i}")
        for i, n in enumerate(CHUNK_ROWS)
    ]

    def load_chunk(i):
        nrows = CHUNK_ROWS[i]
        for dram, sbuf in ((t["xt"], xt_ch[i]), (t["mt"], mt_ch[i])):
            srcap = bass.AP(
                dram,
                _CH_BASE[i] * C * N,
                [[N, 128], [C * N, nrows], [1, N]],
            )
            nc.sync.dma_start(
                sbuf.rearrange("p (r x) -> p r x", r=nrows), srcap
            )

    load_chunk(0)

    # fblob + nbT after chunk 0 on the same SP queue: the DMA engines are
    # shared, so putting these on the ACT DGE queue just interleaves their
    # packets with chunk 0's and delays row 0 by ~3us.
    fblob_sb = const.tile([128, 1 + 2 * RPC], F32, name="fblob_sb")
    nc.sync.dma_start(fblob_sb, t["fblob"].ap())
    bgn_sb = fblob_sb[:, 0:1]
    bias_sb = fblob_sb[:, 1 : 1 + 2 * RPC]

    # nbT split into kc halves so enb(kc0) lands/exps before chunk 1
    nbt_sb = const.tile([128, 2 * H * N], BF16, name="nbt_sb")
    nc.sync.dma_start(
        nbt_sb[:, 0 : H * N],
        bass.AP(t["nbT"], 0, [[2 * H * N, 128], [1, H * N]]),
    )
    nc.sync.dma_start(
        nbt_sb[:, H * N : 2 * H * N],
        bass.AP(t["nbT"], H * N, [[2 * H * N, 128], [1, H * N]]),
    )

    for i in range(1, len(CHUNK_ROWS)):
        load_chunk(i)

    if WITH_BO:
        bo_sb = const.tile([1, C], BF16, name="bo_sb")
        nc.sync.dma_start(bo_sb, t["bo_row"].ap())

    # one-time: exp(nonbatched_bias), k-major layout [p, kc*1024 + h*256 + q]
    # kc0 half exp'd immediately; kc1 half deferred into slot 1 (between
    # exp(0,0) and exp(0,1)) so it doesn't gate row 0's first exp.
    enb_sb = const.tile([128, 2 * H * N], BF16)
    nc.scalar.activation(enb_sb[:, 0:1024], nbt_sb[:, 0:1024], Act.Exp)

    def emit_enb1():
        nc.scalar.activation(
            enb_sb[:, 1024:2048], nbt_sb[:, 1024:2048], Act.Exp
        )

    # ---- PSUM: fixed 8-bank map ----
    ps = ctx.enter_context(tc.tile_pool(name="ps", bufs=1, space="PSUM"))
    lg_t = ps.tile([128, 4 * NB], F32, name="lg_t")  # B0-3: 4 x 256-col res
    wa_t = ps.tile([128, N], F32, name="wa_t", padded_shape=[128, NB])  # B4
    s_t = ps.tile([128, N], F32, name="s_t", padded_shape=[128, NB])  # B5
    px_t = ps.tile([128, 2 * NB], F32, name="px_t")  # B6-7 scratch

    # ---- SBUF working tiles ----
    sbq = ctx.enter_context(tc.tile_pool(name="sbq", bufs=4))
    sb3 = ctx.enter_context(tc.tile_pool(name="sb3", bufs=7))
    sbe = ctx.enter_context(tc.tile_pool(name="sbe", bufs=6))
    sbw = ctx.enter_context(tc.tile_pool(name="sbw", bufs=G // 2 + 5))
    nrow = RPC
    qk_sb = {}  # u -> [128, 4*N] bf16 (q_a | q_b | k_a | k_b)
    v_sb = [None] * nrow  # [128, 2*C] bf16
    e1_sb = {}  # u -> [128, 2*N] f32 gate exp for rows (2u, 2u+1)
    e_sb = {}  # (r, kc) -> [128, H*N] bf16
    wa_sb = {}  # u -> [128, 2*N] bf16 (rows 2u, 2u+1)
    wag_sb = {}  # u -> [128, 2*N] bf16
    d_bat = [
        const.tile([128, G * N], F32, name=f"d_bat{i}") for i in range(2)
    ]
    ln_bat = const.tile([128, G * N], F32, name="ln_bat")
    rs_bat = [
        const.tile([128, G * N], BF16, name=f"rs_bat{i}") for i in range(2)
    ]
    ostage = const.tile([128, 4 * N], BF16, name="ostage")  # 4 rows of out

    def xt_pair(r):
        ci, off = _CH_OF[r]
        return xt_ch[ci].rearrange(
            "p (r x) -> p r x", r=CHUNK_ROWS[ci] // 2
        )[:, off // 2, :]

    def mt_pair(r):
        ci, off = _CH_OF[r]
        return mt_ch[ci].rearrange(
            "p (r x) -> p r x", r=CHUNK_ROWS[ci] // 2
        )[:, off // 2, :]

    def mt_row(r):
        ci, off = _CH_OF[r]
        return mt_ch[ci].rearrange(
            "p (r x) -> p r x", r=CHUNK_ROWS[ci]
        )[:, off, :]

    out_dram = t["out"]

    # ---------------- per-stage emitters ----------------
    # superslot u: projections rows (2u, 2u+1); logits+exp rows (2u-2,
    # 2u-1); waU/S accumulation rows (2u-4, 2u-3).
    def st_wag(u):
        # wag = wa * rs for both rows of superslot u in one op, on GPSIMD
        # (issued as soon as rs is ready, never on the out-pop path).
        wag_sb[u] = sbw.tile([128, 2 * N], BF16, tag="wag", name=f"wag{u}")
        r0 = 2 * u
        nc.vector.tensor_mul(
            wag_sb[u],
            wa_sb[u],
            rs_bat[(r0 // G) % 2][:, (r0 % G) * N : (r0 % G) * N + 2 * N],
        )
        wa_sb[u] = None

    def st_out_mm(u):
        # c-major out pair: out[c, (rr,q)] = wo @ wag-pair, ONE 512-col MM
        # into a full px bank; bo added via a 1-partition broadcast matmul.
        bank = (u % 2) * NB
        MM(
            px_t[:, bank : bank + 2 * N],
            lhsT=wo_sb,
            rhs=wag_sb[u],
            start=True,
            stop=not WITH_BO,
        )
        if WITH_BO:
            MM(
                px_t[:, bank : bank + 2 * N],
                lhsT=bo_sb,
                rhs=ones512_sb,
                start=False,
                stop=True,
                skip_group_check=True,
            )
        wag_sb[u] = None

    def st_out_drain(u):
        # emitted separately so the ACT copy lands BETWEEN exps on the
        # scalar queue, filling the lg-bank WAR ring stall before each exp.
        # Alternate ACT/DVE by parity to balance engine load (ACT ~121us
        # vs DVE ~110us busy otherwise).
        bank = (u % 2) * NB
        half = ostage[:, (u % 2) * 2 * N : (u % 2) * 2 * N + 2 * N]
        if u % 2 == 0:
            nc.scalar.copy(half, px_t[:, bank : bank + 2 * N])
        else:
            nc.vector.tensor_copy(half, px_t[:, bank : bank + 2 * N])
        dst = bass.AP(
            out_dram,
            u * 2 * C * N,
            [[N, 128], [C * N, 2], [1, N]],
        )
        nc.sync.dma_start(dst, half.rearrange("p (r x) -> p r x", r=2))

    def st_qk(u):
        # 2-row projections: q for rows (2u,2u+1) fills B6, k fills B7
        a = 2 * u
        MM(px_t[:, 0:NB], lhsT=wq_sb, rhs=xt_pair(a), start=True, stop=True)
        MM(px_t[:, NB : 2 * NB], lhsT=wk_sb, rhs=mt_pair(a), start=True, stop=True)
        qk_sb[u] = sbq.tile([128, 4 * N], BF16, tag="qk", name=f"qk{u}")
        nc.vector.tensor_copy(qk_sb[u], px_t)

    def st_v(r):
        MM(px_t[:, 0:C], lhsT=mt_row(r)[:, 0:128], rhs=wv_sb, start=True, stop=True)
        MM(
            px_t[:, NB : NB + C],
            lhsT=mt_row(r)[:, 128:256],
            rhs=wv_sb,
            start=True,
            stop=True,
        )
        v_sb[r] = sb3.tile([128, 2 * C], BF16, tag="v", name=f"v{r}")
        nc.vector.tensor_copy(
            v_sb[r].rearrange("p (b x) -> p b x", b=2),
            px_t.rearrange("p (b x) -> p b x", b=2)[:, :, 0:C],
        )

    def st_g(u):
        # 2-row gate projection fills B6 [g_a | g_b]
        MM(px_t[:, 0:NB], lhsT=wg_sb, rhs=xt_pair(2 * u), start=True, stop=True)

    def st_e1(u):
        # e1 = exp(-(gpre+bg)) for both rows, one ACT call from PSUM
        e1_sb[u] = sb3.tile([128, 2 * N], F32, tag="ge1", name=f"ge1_{u}")
        nc.scalar.activation(
            e1_sb[u], px_t[:, 0:NB], Act.Exp, bias=bgn_sb, scale=-1.0
        )

    def st_lg(r, kc):
        # logitsT[ktok, q] = k_h @ q_h.T, 4 heads row-tiled -> B0..B3.
        # (PSUM matmul outputs must be bank-aligned on this build — a
        # half-bank col-offset layout hard-faults at runtime — so lg is
        # single-buffered in banks 0-3 and lg(r,kc') serializes on
        # exp(r,kc) having drained them.)
        qk_t = qk_sb[r // 2]
        qoff = (r % 2) * N
        koff = 2 * N + (r % 2) * N
        for h in range(H):
            MM(
                lg_t[:, NB * h : NB * h + N],
                lhsT=qk_t[
                    32 * h : 32 * h + 32, koff + 128 * kc : koff + 128 * kc + 128
                ],
                rhs=qk_t[32 * h : 32 * h + 32, qoff : qoff + N],
                start=True,
                stop=True,
                tile_position=(32 * h, 0),
            )
        # one exp for all 4 heads; bias col per (kc, r); then *exp(nb)
        et = sbe.tile([128, H * N], BF16, tag=f"e{kc}", name=f"e{kc}_{r}")
        e_sb[(r, kc)] = et
        nc.scalar.activation(
            et.rearrange("p (b x) -> p b x", b=4),
            lg_t.rearrange("p (b x) -> p b x", b=4)[:, :, 0:N],
            Act.Exp,
            bias=bias_sb[:, kc * RPC + r : kc * RPC + r + 1],
            scale=KEY_SCALE,
        )
        nc.vector.tensor_mul(
            et,
            et,
            enb_sb[:, 1024 * kc : 1024 * kc + 1024],
        )

    def et_head(et, h):
        off = h * N
        return et[:, off : off + N]

    def st_waS(r, kc):
        # waU += v_h.T @ e_h (col-tiled by head into B4);  S += 1.T @ e_h (B5)
        et = e_sb[(r, kc)]
        for h in range(H):
            MM(
                wa_t[32 * h : 32 * h + 32, :],
                lhsT=v_sb[r][:, 128 * kc + 32 * h : 128 * kc + 32 * h + 32],
                rhs=et_head(et, h),
                start=(kc == 0),
                stop=(kc == 1),
                tile_position=(0, 32 * h),
                skip_group_check=True,
            )
        for h in range(H):
            MM(
                s_t[32 * h : 32 * h + 32, :],
                lhsT=ones32_sb,
                rhs=et_head(et, h),
                start=(kc == 0),
                stop=(kc == 1),
                tile_position=(0, 32 * h),
                skip_group_check=True,
            )
        e_sb[(r, kc)] = None
        if kc == 1:
            v_sb[r] = None
            # d = (1+e1) * S first (it feeds the batched ln on ACT — the
            # latency-critical path), then drain waU into the pair tile.
            u = r // 2
            nc.vector.scalar_tensor_tensor(
                d_bat[(r // G) % 2][:, (r % G) * N : (r % G) * N + N],
                e1_sb[u][:, (r % 2) * N : (r % 2) * N + N],
                1.0,
                s_t,
                mybir.AluOpType.add,
                mybir.AluOpType.mult,
            )
            if r % 2 == 0:
                wa_sb[u] = sbw.tile(
                    [128, 2 * N], BF16, tag="wa", name=f"wa{u}"
                )
            nc.vector.tensor_copy(
                wa_sb[u][:, (r % 2) * N : (r % 2) * N + N], wa_t
            )

    def st_lnrs(rlast):
        # batched ln + reciprocal-exp for rows [rlast-G+1, rlast]
        nc.scalar.activation(ln_bat, d_bat[(rlast // G) % 2], Act.Ln)
        nc.scalar.activation(
            rs_bat[(rlast // G) % 2], ln_bat, Act.Exp, scale=-1.0
        )

    # PE warmup burst on memset tiles (no DMA dependency): ramps the HAM
    # clock gate while the framework preamble + input DMAs run.
    for w in range(8):
        MM(
            lg_t[0:DH, (w % 4) * NB : (w % 4) * NB + N],
            lhsT=ones32_sb,
            rhs=warm_sb,
            start=True,
            stop=True,
        )

    # ---------------- the software-pipelined superslot loop ----------------
    # slot u: projections rows (2u, 2u+1); logits+exp rows lp=(2u-2, 2u-1);
    # waU/S rows (2u-4, 2u-3).  B4/B5 hold ONE open accumulation group:
    #   slot u: close(2u-5), open(2u-4), close(2u-4), open(2u-3)
    NU = RPC // 2
    out_q = []  # superslots with wag ready, waiting for out stage
    for u in range(NU + 4 + G // 2 + 2):
        a, b = 2 * u, 2 * u + 1  # projection rows this slot
        la, lb = 2 * u - 2, 2 * u - 1  # logits+exp rows
        wp, wq_ = 2 * u - 4, 2 * u - 3  # waU/S rows
        batch_ends = []

        def close_row(r):
            st_waS(r, 1)
            if r % G == G - 1:
                batch_ends.append(r)

        pops = []
        while out_q and len(pops) < 2:
            up = out_q.pop(0)
            st_out_mm(up)
            pops.append(up)
        if 0 <= la < RPC:
            st_lg(la, 0)
        if u == 1:
            emit_enb1()  # deferred kc1 exp(nb): after exp(0,0) on ACT
        if pops:
            st_out_drain(pops[0])  # ACT filler between exp(la,0)/(la,1)
        if 0 <= wp - 1 < RPC and u >= 1:
            close_row(wp - 1)  # row 2u-5
        if 0 <= la < RPC:
            st_lg(la, 1)
        if len(pops) > 1:
            st_out_drain(pops[1])  # ACT filler between exp(la,1)/(lb,0)
        if u < NU:
            st_qk(u)
        if 0 <= wp < RPC:
            st_waS(wp, 0)
        if 0 <= lb < RPC:
            st_lg(lb, 0)
        if u == 0:
            st_g(u)
            st_e1(u)
        if u < NU:
            st_v(a)
            st_v(b)
        if 0 <= wp < RPC:
            close_row(wp)  # row 2u-4
        if 0 < u < NU:
            st_g(u)
            st_e1(u)  # ACT filler between exp(lb,0)/(lb,1)
        if 0 <= lb < RPC:
            st_lg(lb, 1)
        if 0 <= wq_ < RPC:
            st_waS(wq_, 0)
        for be in batch_ends:
            st_lnrs(be)  # ACT filler into next slot's exp(la,0)
            for up in range((be - G + 1) // 2, (be + 1) // 2):
                st_wag(up)
                out_q.append(up)


def _build():
    if "nc" in _CACHE:
        return _CACHE["nc"], _CACHE["t"]
    nc = bass.Bass(
        "TRN2", target_bir_lowering=False, debug=False, num_devices=NCORES
    )
    t = {}
    t["xt"] = nc.dram_tensor("xt", [RPC, C, N], BF16, kind="ExternalInput")
    t["mt"] = nc.dram_tensor("mt", [RPC, C, N], BF16, kind="ExternalInput")
    t["wblob"] = nc.dram_tensor("wblob", [128, 5 * C], BF16, kind="ExternalInput")
    t["fblob"] = nc.dram_tensor("fblob", [128, 1 + 2 * RPC], F32, kind="ExternalInput")
    t["nbT"] = nc.dram_tensor("nbT", [128, 2 * H * N], BF16, kind="ExternalInput")
    if WITH_BO:
        t["bo_row"] = nc.dram_tensor("bo_row", [1, C], BF16, kind="ExternalInput")
    t["out"] = nc.dram_tensor("out", [RPC, C, N], BF16, kind="ExternalOutput")

    with tile.TileContext(nc) as tc:
        with ExitStack() as ctx:
            _emit(ctx, tc, t)
    _legalize_multiwaits(nc, max_waits=1)
    _CACHE["nc"] = nc
    _CACHE["t"] = t
    return nc, t


def _prep_in_maps(q_data, m_data, bias, nonbatched_bias, wq, wk, wv, wo, bo, wg, bg):
    bf16 = mybir.dt.np(BF16)
    q_data = np.ascontiguousarray(np.asarray(q_data, np.float32))
    m_data = np.ascontiguousarray(np.asarray(m_data, np.float32))
    bias = np.asarray(bias, np.float32)
    nb = np.asarray(nonbatched_bias, np.float32)

    # pure layout prep (transposes/reshapes); all math stays on device
    wblob = np.concatenate(
        [
            np.asarray(w, np.float32).T.astype(bf16)
            for w in (wq, wk, wv, wg, wo)
        ],
        axis=1,
    )
    consts = {
        "wblob": np.ascontiguousarray(wblob),
        # nbT[p, kc*1024 + j*256 + q] = nb[0, h, q, kc*128+p] with the
        # bank-major head order j: [h0, h2, h1, h3] (matches et layout)
        "nbT": np.ascontiguousarray(
            nb[0]
            .transpose(2, 0, 1)  # [k, h, q]
            .reshape(2, 128, H, N)
            .transpose(1, 0, 2, 3)
            .reshape(128, 2 * H * N)
            .astype(bf16)
        ),
    }
    if WITH_BO:
        consts["bo_row"] = np.ascontiguousarray(
            np.asarray(bo, np.float32)[None, :].astype(bf16)
        )
    bgn_col = (-np.asarray(bg, np.float32))[:, None]
    # bias_r[p, kc*RPC + r] = bias[0, n0+r, 0, 0, kc*128+p]
    bias_kn = bias[0, :, 0, 0, :].T.reshape(2, 128, N)  # [kc, p, n]
    in_maps = []
    for c in range(NCORES):
        n0 = c * RPC
        rows = slice(n0, n0 + RPC)
        m = dict(consts)
        m["xt"] = np.ascontiguousarray(q_data[0, rows].transpose(0, 2, 1).astype(bf16))
        m["mt"] = np.ascontiguousarray(m_data[0, rows].transpose(0, 2, 1).astype(bf16))
        bias_r = bias_kn[:, :, rows].transpose(1, 0, 2).reshape(128, 2 * RPC)
        m["fblob"] = np.ascontiguousarray(
            np.concatenate([bgn_col, bias_r], axis=1, dtype=np.float32)
        )
        in_maps.append(m)
    return in_maps


def kernel(**inputs) -> np.ndarray:
    global WITH_BO
    want_bo = bool(np.any(np.asarray(inputs["bo"]) != 0))
    if want_bo != WITH_BO or "nc" not in _CACHE:
        WITH_BO = want_bo
        _CACHE.clear()
    nc, _ = _build()
    in_maps = _prep_in_maps(**inputs)
    res = run_bass_kernel_spmd(nc, in_maps, core_ids=list(range(NCORES)))
    out = np.concatenate(
        [
            res.results[c]["out"].astype(np.float32).transpose(0, 2, 1)
            for c in range(NCORES)
        ],
        axis=0,
    )
    return np.ascontiguousarray(out.reshape(B, N, N, C).astype(np.float32))


if __name__ == "__main__":
    # smoke test against a tiny numpy reference
    rng = np.random.default_rng(0)
    inputs = {
        "q_data": rng.standard_normal((B, N, N, C)).astype(np.float32),
        "m_data": rng.standard_normal((B, N, N, C)).astype(np.float32),
        "bias": rng.standard_normal((B, N, 1, 1, N)).astype(np.float32),
        "nonbatched_bias": rng.standard_normal((1, H, N, N)).astype(np.float32),
        "wq": (rng.standard_normal((C, C)) / np.sqrt(C)).astype(np.float32),
        "wk": (rng.standard_normal((C, C)) / np.sqrt(C)).astype(np.float32),
        "wv": (rng.standard_normal((C, C)) / np.sqrt(C)).astype(np.float32),
        "wo": (rng.standard_normal((C, C)) / np.sqrt(C)).astype(np.float32),
        "bo": np.zeros((C,), np.float32),
        "wg": np.ones((C, C), np.float32) / np.sqrt(C),
        "bg": np.ones((C,), np.float32),
    }
    out = kernel(**inputs)
    print("out", out.shape, out.dtype, float(np.abs(out).max()))


# revision 51
# speedup vs baseline: 1.1906x; 1.1906x over previous
"""Trainium2 Bass kernel for gated multi-head pair attention (AlphaFold-style).

Reference computation (B=1, N=256, C=128, H=4, DH=32):
    q = (q_data @ wq.T) * DH**-0.5        # [B,N,Nq,C]
    k = m_data @ wk.T ; v = m_data @ wv.T
    logits = einsum("bnqhd,bnkhd->bnhqk", q, k) + bias + nonbatched_bias
    weight = softmax(logits, axis=-1)
    wa = einsum("bnhqk,bnkhd->bnqhd", weight, v)
    g  = sigmoid(q_data @ wg.T + bg)
    out = (wa * g).reshape(...) @ wo.T + bo

Sharding: pure data-parallel across the 8 NeuronCores along the first
residue axis (N): core c owns rows [32c, 32c+32). Params + nonbatched_bias
replicated.

v4 (four-engine pipeline): k-major like v2/v3, restructured around the
measured per-engine workload (ACT was the binding engine at ~3.45us/row):
  - four-engine split: ACT keeps only exp/ln work (et-exps, gate-exp,
    batched ln+rs).  GPSIMD (idle before) takes the kc0 et*exp(nb)
    multiply and the wag=wa*rs multiply (SBUF-only tensor ops DO compile
    on this build; PSUM access does not).  DVE keeps the PSUM drains, the
    kc1 multiply, d=(1+e1)*S, and the out staging copy.
  - deeper software pipeline: logits+exp for rows (2u-2, 2u-1) and
    waU/S accumulation for rows (2u-4, 2u-3) run in superslot u, so the
    slow GPSIMD multiply (~2.1us) is off the PE's in-order critical path.
  - wag is computed at rs-batch completion time (not at out-pop time) so
    the out matmul never waits on GPSIMD latency.
  - DMA: consts packed into two blobs; input DMAs split across the SP and
    ACT hardware DGE queues (fblob+nbT on ACT, xt/mt chunks on SP) so the
    nonbatched-bias transfer doesn't serialize behind the input chunks;
    first chunk is only 2 rows so row 0 can start ~4us earlier.
  - exp(nb) is loaded bf16 and exp'd once; bias folded into the et-exp's
    per-partition bias port; softmax denom and sigmoid gate fused into
    ONE reciprocal chain rs = exp(-ln((1+e1)*S)).

Environment notes (this walrus build): one sem wait max per instruction
(_legalize_multiwaits); two matmuls must never concurrently target
different column ranges of the same PSUM bank; gpsimd cannot access PSUM;
no PSUM-source DMAs; only exp/ln ACT funcs are used so the ACT table set
loads exactly once.  Measured: the 4 tile-positioned matmuls of a group
stream their output columns serially (cost = total cols at 2.4GHz);
lg-bank kc double-buffering is SLOWER (PSUM port contention with the
exp's reads); gpsimd Multiply eff=0.42 -> [128,1024] mul ~2.1us.
"""

import os
import sys

sys.path.insert(0, "/opt/trn_rl_repo")

from contextlib import ExitStack

import numpy as np

import concourse.bass as bass
import concourse.tile as tile
from concourse import mybir
from concourse.bass_utils import run_bass_kernel_spmd

B, N, C, H = 1, 256, 128, 4
DH = C // H
KEY_SCALE = DH**-0.5
NCORES = 8
RPC = N // NCORES  # rows per core
G = 4  # rows per batched ln/rs epilogue call
CHUNK_ROWS = [2, 6, 8, 8, 8]  # uneven input chunks: row 0 starts early

F32 = mybir.dt.float32
BF16 = mybir.dt.bfloat16

WITH_BO = True  # set by kernel() per-input; bo==0 skips the bias matmuls

_CACHE = {}


def _legalize_multiwaits(nc, max_waits=1):
    """The walrus build here encodes at most one sem wait per instruction
    ("Too many sync wait commands" otherwise). Split excess waits onto
    freshly inserted Drain instructions on the same engine just before the
    multi-wait instruction (engines execute in order, so this is
    equivalent)."""
    n_fix = 0
    for f in nc.m.functions:
        for blk in f.blocks:
            changed = False
            new_insts = []
            for inst in blk.instructions:
                si = inst.sync_info
                ow = list(si.on_wait) if (si is not None and si.on_wait) else []
                if len(ow) > max_waits:
                    head, tail = ow[:-max_waits], ow[-max_waits:]
                    while head:
                        chunk, head = head[:max_waits], head[max_waits:]
                        d = mybir.InstNoOp(
                            name=f"I-mw{nc.next_id()}", ins=[], outs=[]
                        )
                        d.engine = inst.engine
                        d.sync_info = mybir.SyncInfo(
                            on_wait=list(chunk), on_update=[]
                        )
                        new_insts.append(d)
                        n_fix += 1
                    inst.sync_info = mybir.SyncInfo(
                        on_wait=list(tail),
                        on_update=list(si.on_update) if si.on_update else [],
                    )
                    changed = True
                new_insts.append(inst)
            if changed:
                blk.instructions = new_insts
    return n_fix


# row -> (chunk index, row offset inside chunk)
_CH_OF = []
for _ci, _n in enumerate(CHUNK_ROWS):
    for _j in range(_n):
        _CH_OF.append((_ci, _j))
_CH_BASE = [sum(CHUNK_ROWS[:i]) for i in range(len(CHUNK_ROWS))]


def _emit(ctx: ExitStack, tc: "tile.TileContext", t):
    nc = tc.nc
    MM = nc.tensor.matmul
    Act = mybir.ActivationFunctionType
    NB = 512  # psum bank stride (fp32 elems)

    const = ctx.enter_context(tc.tile_pool(name="const", bufs=1))

    # warmup rhs + ones tiles first: memset-only (no DMA dependency) so the
    # PE ramp burst can start immediately.
    warm_sb = const.tile([128, N], BF16, name="warm_sb")
    nc.vector.memset(warm_sb, 0.0)
    ones32_sb = const.tile([128, DH], BF16)
    nc.vector.memset(ones32_sb, 1.0)
    if WITH_BO:
        ones512_sb = const.tile([1, 2 * N], BF16)
        nc.vector.memset(ones512_sb, 1.0)

    # consolidated const DMAs, ordered for the row-0 critical path on the
    # SP queue: wblob -> xt/mt chunk 0 -> remaining chunks.  fblob + nbT
    # go down the ACT engine's independent HW DGE queue in parallel.
    wblob_sb = const.tile([128, 5 * C], BF16, name="wblob_sb")
    nc.sync.dma_start(wblob_sb, t["wblob"].ap())
    wq_sb = wblob_sb[:, 0 * C : 1 * C]
    wk_sb = wblob_sb[:, 1 * C : 2 * C]
    wv_sb = wblob_sb[:, 2 * C : 3 * C]
    wg_sb = wblob_sb[:, 3 * C : 4 * C]
    wo_sb = wblob_sb[:, 4 * C : 5 * C]

    xt_ch = [
        const.tile([128, n * N], BF16, name=f"xt_ch{i}")
        for i, n in enumerate(CHUNK_ROWS)
    ]
    mt_ch = [
        const.tile([128, n * N], BF16, name=f"mt_ch{i}")
        for i, n in enumerate(CHUNK_ROWS)
    ]

    def load_chunk(i):
        nrows = CHUNK_ROWS[i]
        for dram, sbuf in ((t["xt"], xt_ch[i]), (t["mt"], mt_ch[i])):
            srcap = bass.AP(
                dram,
                _CH_BASE[i] * C * N,
                [[N, 128], [C * N, nrows], [1, N]],
            )
            nc.sync.dma_start(
                sbuf.rearrange("p (r x) -> p r x", r=nrows), srcap
            )

    load_chunk(0)

    # fblob + nbT after chunk 0 on the same SP queue: the DMA engines are
    # shared, so putting these on the ACT DGE queue just interleaves their
    # packets with chunk 0's and delays row 0 by ~3us.
    fblob_sb = const.tile([128, 1 + 2 * RPC], F32, name="fblob_sb")
    nc.sync.dma_start(fblob_sb, t["fblob"].ap())
    bgn_sb = fblob_sb[:, 0:1]
    bias_sb = fblob_sb[:, 1 : 1 + 2 * RPC]

    # nbT split into kc halves so enb(kc0) lands/exps before chunk 1
    nbt_sb = const.tile([128, 2 * H * N], BF16, name="nbt_sb")
    nc.sync.dma_start(
        nbt_sb[:, 0 : H * N],
        bass.AP(t["nbT"], 0, [[2 * H * N, 128], [1, H * N]]),
    )
    nc.sync.dma_start(
        nbt_sb[:, H * N : 2 * H * N],
        bass.AP(t["nbT"], H * N, [[2 * H * N, 128], [1, H * N]]),
    )

    for i in range(1, len(CHUNK_ROWS)):
        load_chunk(i)

    if WITH_BO:
        bo_sb = const.tile([1, C], BF16, name="bo_sb")
        nc.sync.dma_start(bo_sb, t["bo_row"].ap())

    # one-time: exp(nonbatched_bias), k-major layout [p, kc*1024 + h*256 + q]
    # kc0 half exp'd immediately; kc1 half deferred into slot 1 (between
    # exp(0,0) and exp(0,1)) so it doesn't gate row 0's first exp.
    enb_sb = const.tile([128, 2 * H * N], BF16)
    nc.scalar.activation(enb_sb[:, 0:1024], nbt_sb[:, 0:1024], Act.Exp)

    def emit_enb1():
        nc.scalar.activation(
            enb_sb[:, 1024:2048], nbt_sb[:, 1024:2048], Act.Exp
        )

    # ---- PSUM: fixed 8-bank map ----
    ps = ctx.enter_context(tc.tile_pool(name="ps", bufs=1, space="PSUM"))
    lg_t = ps.tile([128, 4 * NB], F32, name="lg_t")  # B0-3: 4 x 256-col res
    wa_t = ps.tile([128, N], F32, name="wa_t", padded_shape=[128, NB])  # B4
    s_t = ps.tile([128, N], F32, name="s_t", padded_shape=[128, NB])  # B5
    px_t = ps.tile([128, 2 * NB], F32, name="px_t")  # B6-7 scratch

    # ---- SBUF working tiles ----
    sbq = ctx.enter_context(tc.tile_pool(name="sbq", bufs=4))
    sb3 = ctx.enter_context(tc.tile_pool(name="sb3", bufs=7))
    sbe = ctx.enter_context(tc.tile_pool(name="sbe", bufs=6))
    sbw = ctx.enter_context(tc.tile_pool(name="sbw", bufs=G // 2 + 5))
    nrow = RPC
    qk_sb = {}  # u -> [128, 4*N] bf16 (q_a | q_b | k_a | k_b)
    v_sb = [None] * nrow  # [128, 2*C] bf16
    e1_sb = {}  # u -> [128, 2*N] f32 gate exp for rows (2u, 2u+1)
    e_sb = {}  # (r, kc) -> [128, H*N] bf16
    wa_sb = {}  # u -> [128, 2*N] bf16 (rows 2u, 2u+1)
    wag_sb = {}  # u -> [128, 2*N] bf16
    d_bat = [
        const.tile([128, G * N], F32, name=f"d_bat{i}") for i in range(2)
    ]
    ln_bat = const.tile([128, G * N], F32, name="ln_bat")
    rs_bat = [
        const.tile([128, G * N], BF16, name=f"rs_bat{i}") for i in range(2)
    ]
    ostage = const.tile([128, 4 * N], BF16, name="ostage")  # 4 rows of out

    def xt_pair(r):
        ci, off = _CH_OF[r]
        return xt_ch[ci].rearrange(
            "p (r x) -> p r x", r=CHUNK_ROWS[ci] // 2
        )[:, off // 2, :]

    def mt_pair(r):
        ci, off = _CH_OF[r]
        return mt_ch[ci].rearrange(
            "p (r x) -> p r x", r=CHUNK_ROWS[ci] // 2
        )[:, off // 2, :]

    def mt_row(r):
        ci, off = _CH_OF[r]
        return mt_ch[ci].rearrange(
            "p (r x) -> p r x", r=CHUNK_ROWS[ci]
        )[:, off, :]

    out_dram = t["out"]

    # ---------------- per-stage emitters ----------------
    # superslot u: projections rows (2u, 2u+1); logits+exp rows (2u-2,
    # 2u-1); waU/S accumulation rows (2u-4, 2u-3).
    def st_wag(u):
        # wag = wa * rs for both rows of superslot u in one op, on GPSIMD
        # (issued as soon as rs is ready, never on the out-pop path).
        wag_sb[u] = sbw.tile([128, 2 * N], BF16, tag="wag", name=f"wag{u}")
        r0 = 2 * u
        nc.vector.tensor_mul(
            wag_sb[u],
            wa_sb[u],
            rs_bat[(r0 // G) % 2][:, (r0 % G) * N : (r0 % G) * N + 2 * N],
        )
        wa_sb[u] = None

    def st_out_mm(u):
        # c-major out pair: out[c, (rr,q)] = wo @ wag-pair, ONE 512-col MM
        # into a full px bank; bo added via a 1-partition broadcast matmul.
        bank = (u % 2) * NB
        MM(
            px_t[:, bank : bank + 2 * N],
            lhsT=wo_sb,
            rhs=wag_sb[u],
            start=True,
            stop=not WITH_BO,
        )
        if WITH_BO:
            MM(
                px_t[:, bank : bank + 2 * N],
                lhsT=bo_sb,
                rhs=ones512_sb,
                start=False,
                stop=True,
                skip_group_check=True,
            )
        wag_sb[u] = None

    def st_out_drain(u):
        # emitted separately so the ACT copy lands BETWEEN exps on the
        # scalar queue, filling the lg-bank WAR ring stall before each exp.
        # (Putting this on DVE instead was measured MUCH slower: the copy
        # lands deep in the DVE queue and its px read stalls the next
        # slot's matmuls.)
        bank = (u % 2) * NB
        half = ostage[:, (u % 2) * 2 * N : (u % 2) * 2 * N + 2 * N]
        nc.scalar.copy(half, px_t[:, bank : bank + 2 * N])
        dst = bass.AP(
            out_dram,
            u * 2 * C * N,
            [[N, 128], [C * N, 2], [1, N]],
        )
        nc.sync.dma_start(dst, half.rearrange("p (r x) -> p r x", r=2))

    def st_qk(u):
        # 2-row projections: q for rows (2u,2u+1) fills B6, k fills B7
        a = 2 * u
        MM(px_t[:, 0:NB], lhsT=wq_sb, rhs=xt_pair(a), start=True, stop=True)
        MM(px_t[:, NB : 2 * NB], lhsT=wk_sb, rhs=mt_pair(a), start=True, stop=True)
        qk_sb[u] = sbq.tile([128, 4 * N], BF16, tag="qk", name=f"qk{u}")
        nc.vector.tensor_copy(qk_sb[u], px_t)

    def st_v(r):
        MM(px_t[:, 0:C], lhsT=mt_row(r)[:, 0:128], rhs=wv_sb, start=True, stop=True)
        MM(
            px_t[:, NB : NB + C],
            lhsT=mt_row(r)[:, 128:256],
            rhs=wv_sb,
            start=True,
            stop=True,
        )
        v_sb[r] = sb3.tile([128, 2 * C], BF16, tag="v", name=f"v{r}")
        nc.vector.tensor_copy(
            v_sb[r].rearrange("p (b x) -> p b x", b=2),
            px_t.rearrange("p (b x) -> p b x", b=2)[:, :, 0:C],
        )

    def st_g(u):
        # 2-row gate projection fills B6 [g_a | g_b]
        MM(px_t[:, 0:NB], lhsT=wg_sb, rhs=xt_pair(2 * u), start=True, stop=True)

    def st_e1(u):
        # e1 = exp(-(gpre+bg)) for both rows, one ACT call from PSUM
        e1_sb[u] = sb3.tile([128, 2 * N], F32, tag="ge1", name=f"ge1_{u}")
        nc.scalar.activation(
            e1_sb[u], px_t[:, 0:NB], Act.Exp, bias=bgn_sb, scale=-1.0
        )

    def st_lg(r, kc):
        # logitsT[ktok, q] = k_h @ q_h.T, 4 heads row-tiled -> B0..B3.
        # (PSUM matmul outputs must be bank-aligned on this build — a
        # half-bank col-offset layout hard-faults at runtime — so lg is
        # single-buffered in banks 0-3 and lg(r,kc') serializes on
        # exp(r,kc) having drained them.)
        qk_t = qk_sb[r // 2]
        qoff = (r % 2) * N
        koff = 2 * N + (r % 2) * N
        for h in range(H):
            MM(
                lg_t[:, NB * h : NB * h + N],
                lhsT=qk_t[
                    32 * h : 32 * h + 32, koff + 128 * kc : koff + 128 * kc + 128
                ],
                rhs=qk_t[32 * h : 32 * h + 32, qoff : qoff + N],
                start=True,
                stop=True,
                tile_position=(32 * h, 0),
            )
        # one exp for all 4 heads; bias col per (kc, r); then *exp(nb)
        et = sbe.tile([128, H * N], BF16, tag=f"e{kc}", name=f"e{kc}_{r}")
        e_sb[(r, kc)] = et
        nc.scalar.activation(
            et.rearrange("p (b x) -> p b x", b=4),
            lg_t.rearrange("p (b x) -> p b x", b=4)[:, :, 0:N],
            Act.Exp,
            bias=bias_sb[:, kc * RPC + r : kc * RPC + r + 1],
            scale=KEY_SCALE,
        )
        nc.vector.tensor_mul(
            et,
            et,
            enb_sb[:, 1024 * kc : 1024 * kc + 1024],
        )

    def et_head(et, h):
        off = h * N
        return et[:, off : off + N]

    def st_waS(r, kc):
        # waU += v_h.T @ e_h (col-tiled by head into B4);  S += 1.T @ e_h (B5)
        et = e_sb[(r, kc)]
        for h in range(H):
            MM(
                wa_t[32 * h : 32 * h + 32, :],
                lhsT=v_sb[r][:, 128 * kc + 32 * h : 128 * kc + 32 * h + 32],
                rhs=et_head(et, h),
                start=(kc == 0),
                stop=(kc == 1),
                tile_position=(0, 32 * h),
                skip_group_check=True,
            )
        for h in range(H):
            MM(
                s_t[32 * h : 32 * h + 32, :],
                lhsT=ones32_sb,
                rhs=et_head(et, h),
                start=(kc == 0),
                stop=(kc == 1),
                tile_position=(0, 32 * h),
                skip_group_check=True,
            )
        e_sb[(r, kc)] = None
        if kc == 1:
            v_sb[r] = None
            # d = (1+e1) * S first (it feeds the batched ln on ACT — the
            # latency-critical path), then drain waU into the pair tile.
            u = r // 2
            nc.vector.scalar_tensor_tensor(
                d_bat[(r // G) % 2][:, (r % G) * N : (r % G) * N + N],
                e1_sb[u][:, (r % 2) * N : (r % 2) * N + N],
                1.0,
                s_t,
                mybir.AluOpType.add,
                mybir.AluOpType.mult,
            )
            if r % 2 == 0:
                wa_sb[u] = sbw.tile(
                    [128, 2 * N], BF16, tag="wa", name=f"wa{u}"
                )
            nc.vector.tensor_copy(
                wa_sb[u][:, (r % 2) * N : (r % 2) * N + N], wa_t
            )

    def st_lnrs(rlast):
        # batched ln + reciprocal-exp for rows [rlast-G+1, rlast]
        nc.scalar.activation(ln_bat, d_bat[(rlast // G) % 2], Act.Ln)
        nc.scalar.activation(
            rs_bat[(rlast // G) % 2], ln_bat, Act.Exp, scale=-1.0
        )

    # PE warmup burst on memset tiles (no DMA dependency): ramps the HAM
    # clock gate while the framework preamble + input DMAs run.
    for w in range(8):
        MM(
            lg_t[0:DH, (w % 4) * NB : (w % 4) * NB + N],
            lhsT=ones32_sb,
            rhs=warm_sb,
            start=True,
            stop=True,
        )

    # ---------------- the software-pipelined superslot loop ----------------
    # slot u: projections rows (2u, 2u+1); logits+exp rows lp=(2u-2, 2u-1);
    # waU/S rows (2u-4, 2u-3).  B4/B5 hold ONE open accumulation group:
    #   slot u: close(2u-5), open(2u-4), close(2u-4), open(2u-3)
    NU = RPC // 2
    out_q = []  # superslots with wag ready, waiting for out stage
    for u in range(NU + 4 + G // 2 + 2):
        a, b = 2 * u, 2 * u + 1  # projection rows this slot
        la, lb = 2 * u - 2, 2 * u - 1  # logits+exp rows
        wp, wq_ = 2 * u - 4, 2 * u - 3  # waU/S rows
        batch_ends = []

        def close_row(r):
            st_waS(r, 1)
            if r % G == G - 1:
                batch_ends.append(r)

        pops = []
        while out_q and len(pops) < 2:
            up = out_q.pop(0)
            st_out_mm(up)
            pops.append(up)
        if 0 <= la < RPC:
            st_lg(la, 0)
        if u == 1:
            emit_enb1()  # deferred kc1 exp(nb): after exp(0,0) on ACT
        if pops:
            st_out_drain(pops[0])  # ACT filler between exp(la,0)/(la,1)
        if 0 <= wp - 1 < RPC and u >= 1:
            close_row(wp - 1)  # row 2u-5
        if 0 <= la < RPC:
            st_lg(la, 1)
        if len(pops) > 1:
            st_out_drain(pops[1])  # ACT filler between exp(la,1)/(lb,0)
        if u < NU:
            st_qk(u)
        if 0 <= wp < RPC:
            st_waS(wp, 0)
        if 0 <= lb < RPC:
            st_lg(lb, 0)
        if u == 0:
            st_g(u)
            st_e1(u)
        if u < NU:
            st_v(a)
            st_v(b)
        if 0 <= wp < RPC:
            close_row(wp)  # row 2u-4
        if 0 < u < NU:
            st_g(u)
            st_e1(u)  # ACT filler between exp(lb,0)/(lb,1)
        if 0 <= lb < RPC:
            st_lg(lb, 1)
        if 0 <= wq_ < RPC:
            st_waS(wq_, 0)
        for be in batch_ends:
            st_lnrs(be)  # ACT filler into next slot's exp(la,0)
            for up in range((be - G + 1) // 2, (be + 1) // 2):
                st_wag(up)
                out_q.append(up)


def _build():
    if "nc" in _CACHE:
        return _CACHE["nc"], _CACHE["t"]
    nc = bass.Bass(
        "TRN2", target_bir_lowering=False, debug=False, num_devices=NCORES
    )
    t = {}
    t["xt"] = nc.dram_tensor("xt", [RPC, C, N], BF16, kind="ExternalInput")
    t["mt"] = nc.dram_tensor("mt", [RPC, C, N], BF16, kind="ExternalInput")
    t["wblob"] = nc.dram_tensor("wblob", [128, 5 * C], BF16, kind="ExternalInput")
    t["fblob"] = nc.dram_tensor("fblob", [128, 1 + 2 * RPC], F32, kind="ExternalInput")
    t["nbT"] = nc.dram_tensor("nbT", [128, 2 * H * N], BF16, kind="ExternalInput")
    if WITH_BO:
        t["bo_row"] = nc.dram_tensor("bo_row", [1, C], BF16, kind="ExternalInput")
    t["out"] = nc.dram_tensor("out", [RPC, C, N], BF16, kind="ExternalOutput")

    with tile.TileContext(nc) as tc:
        with ExitStack() as ctx:
            _emit(ctx, tc, t)
    _legalize_multiwaits(nc, max_waits=1)
    _CACHE["nc"] = nc
    _CACHE["t"] = t
    return nc, t


def _prep_in_maps(q_data, m_data, bias, nonbatched_bias, wq, wk, wv, wo, bo, wg, bg):
    bf16 = mybir.dt.np(BF16)
    q_data = np.ascontiguousarray(np.asarray(q_data, np.float32))
    m_data = np.ascontiguousarray(np.asarray(m_data, np.float32))
    bias = np.asarray(bias, np.float32)
    nb = np.asarray(nonbatched_bias, np.float32)

    # pure layout prep (transposes/reshapes); all math stays on device
    wblob = np.concatenate(
        [
            np.asarray(w, np.float32).T.astype(bf16)
            for w in (wq, wk, wv, wg, wo)
        ],
        axis=1,
    )
    consts = {
        "wblob": np.ascontiguousarray(wblob),
        # nbT[p, kc*1024 + j*256 + q] = nb[0, h, q, kc*128+p] with the
        # bank-major head order j: [h0, h2, h1, h3] (matches et layout)
        "nbT": np.ascontiguousarray(
            nb[0]
            .transpose(2, 0, 1)  # [k, h, q]
            .reshape(2, 128, H, N)
            .transpose(1, 0, 2, 3)
            .reshape(128, 2 * H * N)
            .astype(bf16)
        ),
    }
    if WITH_BO:
        consts["bo_row"] = np.ascontiguousarray(
            np.asarray(bo, np.float32)[None, :].astype(bf16)
        )
    bgn_col = (-np.asarray(bg, np.float32))[:, None]
    # bias_r[p, kc*RPC + r] = bias[0, n0+r, 0, 0, kc*128+p]
    bias_kn = bias[0, :, 0, 0, :].T.reshape(2, 128, N)  # [kc, p, n]
    in_maps = []
    for c in range(NCORES):
        n0 = c * RPC
        rows = slice(n0, n0 + RPC)
        m = dict(consts)
        m["xt"] = np.ascontiguousarray(q_data[0, rows].transpose(0, 2, 1).astype(bf16))
        m["mt"] = np.ascontiguousarray(m_data[0, rows].transpose(0, 2, 1).astype(bf16))
        bias_r = bias_kn[:, :, rows].transpose(1, 0, 2).reshape(128, 2 * RPC)
        m["fblob"] = np.ascontiguousarray(
            np.concatenate([bgn_col, bias_r], axis=1, dtype=np.float32)
        )
        in_maps.append(m)
    return in_maps


def kernel(**inputs) -> np.ndarray:
    global WITH_BO
    want_bo = bool(np.any(np.asarray(inputs["bo"]) != 0))
    if want_bo != WITH_BO or "nc" not in _CACHE:
        WITH_BO = want_bo
        _CACHE.clear()
    nc, _ = _build()
    in_maps = _prep_in_maps(**inputs)
    res = run_bass_kernel_spmd(nc, in_maps, core_ids=list(range(NCORES)))
    out = np.concatenate(
        [
            res.results[c]["out"].astype(np.float32).transpose(0, 2, 1)
            for c in range(NCORES)
        ],
        axis=0,
    )
    return np.ascontiguousarray(out.reshape(B, N, N, C).astype(np.float32))


if __name__ == "__main__":
    # smoke test against a tiny numpy reference
    rng = np.random.default_rng(0)
    inputs = {
        "q_data": rng.standard_normal((B, N, N, C)).astype(np.float32),
        "m_data": rng.standard_normal((B, N, N, C)).astype(np.float32),
        "bias": rng.standard_normal((B, N, 1, 1, N)).astype(np.float32),
        "nonbatched_bias": rng.standard_normal((1, H, N, N)).astype(np.float32),
        "wq": (rng.standard_normal((C, C)) / np.sqrt(C)).astype(np.float32),
        "wk": (rng.standard_normal((C, C)) / np.sqrt(C)).astype(np.float32),
        "wv": (rng.standard_normal((C, C)) / np.sqrt(C)).astype(np.float32),
        "wo": (rng.standard_normal((C, C)) / np.sqrt(C)).astype(np.float32),
        "bo": np.zeros((C,), np.float32),
        "wg": np.ones((C, C), np.float32) / np.sqrt(C),
        "bg": np.ones((C,), np.float32),
    }
    out = kernel(**inputs)
    print("out", out.shape, out.dtype, float(np.abs(out).max()))


# revision 54
# speedup vs baseline: 1.2026x; 1.0101x over previous
"""Trainium2 Bass kernel for gated multi-head pair attention (AlphaFold-style).

Reference computation (B=1, N=256, C=128, H=4, DH=32):
    q = (q_data @ wq.T) * DH**-0.5        # [B,N,Nq,C]
    k = m_data @ wk.T ; v = m_data @ wv.T
    logits = einsum("bnqhd,bnkhd->bnhqk", q, k) + bias + nonbatched_bias
    weight = softmax(logits, axis=-1)
    wa = einsum("bnhqk,bnkhd->bnqhd", weight, v)
    g  = sigmoid(q_data @ wg.T + bg)
    out = (wa * g).reshape(...) @ wo.T + bo

Sharding: pure data-parallel across the 8 NeuronCores along the first
residue axis (N): core c owns rows [32c, 32c+32). Params + nonbatched_bias
replicated.

v4 (four-engine pipeline): k-major like v2/v3, restructured around the
measured per-engine workload (ACT was the binding engine at ~3.45us/row):
  - four-engine split: ACT keeps only exp/ln work (et-exps, gate-exp,
    batched ln+rs).  GPSIMD (idle before) takes the kc0 et*exp(nb)
    multiply and the wag=wa*rs multiply (SBUF-only tensor ops DO compile
    on this build; PSUM access does not).  DVE keeps the PSUM drains, the
    kc1 multiply, d=(1+e1)*S, and the out staging copy.
  - deeper software pipeline: logits+exp for rows (2u-2, 2u-1) and
    waU/S accumulation for rows (2u-4, 2u-3) run in superslot u, so the
    slow GPSIMD multiply (~2.1us) is off the PE's in-order critical path.
  - wag is computed at rs-batch completion time (not at out-pop time) so
    the out matmul never waits on GPSIMD latency.
  - DMA: consts packed into two blobs; input DMAs split across the SP and
    ACT hardware DGE queues (fblob+nbT on ACT, xt/mt chunks on SP) so the
    nonbatched-bias transfer doesn't serialize behind the input chunks;
    first chunk is only 2 rows so row 0 can start ~4us earlier.
  - exp(nb) is loaded bf16 and exp'd once; bias folded into the et-exp's
    per-partition bias port; softmax denom and sigmoid gate fused into
    ONE reciprocal chain rs = exp(-ln((1+e1)*S)).

Environment notes (this walrus build): one sem wait max per instruction
(_legalize_multiwaits); two matmuls must never concurrently target
different column ranges of the same PSUM bank; gpsimd cannot access PSUM;
no PSUM-source DMAs; only exp/ln ACT funcs are used so the ACT table set
loads exactly once.  Measured: the 4 tile-positioned matmuls of a group
stream their output columns serially (cost = total cols at 2.4GHz);
lg-bank kc double-buffering is SLOWER (PSUM port contention with the
exp's reads); gpsimd Multiply eff=0.42 -> [128,1024] mul ~2.1us.
"""

import os
import sys

sys.path.insert(0, "/opt/trn_rl_repo")

from contextlib import ExitStack

import numpy as np

import concourse.bass as bass
import concourse.tile as tile
from concourse import mybir
from concourse.bass_utils import run_bass_kernel_spmd

B, N, C, H = 1, 256, 128, 4
DH = C // H
KEY_SCALE = DH**-0.5
NCORES = 8
RPC = N // NCORES  # rows per core
G = 4  # rows per batched ln/rs epilogue call
CHUNK_ROWS = [2, 6, 8, 8, 8]  # uneven input chunks: row 0 starts early

F32 = mybir.dt.float32
BF16 = mybir.dt.bfloat16

WITH_BO = True  # set by kernel() per-input; bo==0 skips the bias matmuls

_CACHE = {}


def _legalize_multiwaits(nc, max_waits=1):
    """The walrus build here encodes at most one sem wait per instruction
    ("Too many sync wait commands" otherwise). Split excess waits onto
    freshly inserted Drain instructions on the same engine just before the
    multi-wait instruction (engines execute in order, so this is
    equivalent)."""
    n_fix = 0
    for f in nc.m.functions:
        for blk in f.blocks:
            changed = False
            new_insts = []
            for inst in blk.instructions:
                si = inst.sync_info
                ow = list(si.on_wait) if (si is not None and si.on_wait) else []
                if len(ow) > max_waits:
                    head, tail = ow[:-max_waits], ow[-max_waits:]
                    while head:
                        chunk, head = head[:max_waits], head[max_waits:]
                        d = mybir.InstNoOp(
                            name=f"I-mw{nc.next_id()}", ins=[], outs=[]
                        )
                        d.engine = inst.engine
                        d.sync_info = mybir.SyncInfo(
                            on_wait=list(chunk), on_update=[]
                        )
                        new_insts.append(d)
                        n_fix += 1
                    inst.sync_info = mybir.SyncInfo(
                        on_wait=list(tail),
                        on_update=list(si.on_update) if si.on_update else [],
                    )
                    changed = True
                new_insts.append(inst)
            if changed:
                blk.instructions = new_insts
    return n_fix


# row -> (chunk index, row offset inside chunk)
_CH_OF = []
for _ci, _n in enumerate(CHUNK_ROWS):
    for _j in range(_n):
        _CH_OF.append((_ci, _j))
_CH_BASE = [sum(CHUNK_ROWS[:i]) for i in range(len(CHUNK_ROWS))]


def _emit(ctx: ExitStack, tc: "tile.TileContext", t):
    nc = tc.nc
    MM = nc.tensor.matmul
    Act = mybir.ActivationFunctionType
    NB = 512  # psum bank stride (fp32 elems)

    const = ctx.enter_context(tc.tile_pool(name="const", bufs=1))

    # warmup rhs + ones tiles first: memset-only (no DMA dependency) so the
    # PE ramp burst can start immediately.
    warm_sb = const.tile([128, N], BF16, name="warm_sb")
    nc.vector.memset(warm_sb, 0.0)
    ones32_sb = const.tile([128, DH], BF16)
    nc.vector.memset(ones32_sb, 1.0)
    if WITH_BO:
        ones512_sb = const.tile([1, 2 * N], BF16)
        nc.vector.memset(ones512_sb, 1.0)

    # consolidated const DMAs, ordered for the row-0 critical path on the
    # SP queue: wblob -> xt/mt chunk 0 -> remaining chunks.  fblob + nbT
    # go down the ACT engine's independent HW DGE queue in parallel.
    wblob_sb = const.tile([128, 5 * C], BF16, name="wblob_sb")
    nc.sync.dma_start(wblob_sb, t["wblob"].ap())
    wq_sb = wblob_sb[:, 0 * C : 1 * C]
    wk_sb = wblob_sb[:, 1 * C : 2 * C]
    wv_sb = wblob_sb[:, 2 * C : 3 * C]
    wg_sb = wblob_sb[:, 3 * C : 4 * C]
    wo_sb = wblob_sb[:, 4 * C : 5 * C]

    xt_ch = [
        const.tile([128, n * N], BF16, name=f"xt_ch{i}")
        for i, n in enumerate(CHUNK_ROWS)
    ]
    mt_ch = [
        const.tile([128, n * N], BF16, name=f"mt_ch{i}")
        for i, n in enumerate(CHUNK_ROWS)
    ]

    def load_chunk(i):
        nrows = CHUNK_ROWS[i]
        for dram, sbuf in ((t["xt"], xt_ch[i]), (t["mt"], mt_ch[i])):
            srcap = bass.AP(
                dram,
                _CH_BASE[i] * C * N,
                [[N, 128], [C * N, nrows], [1, N]],
            )
            nc.sync.dma_start(
                sbuf.rearrange("p (r x) -> p r x", r=nrows), srcap
            )

    load_chunk(0)

    # fblob + nbT after chunk 0 on the same SP queue: the DMA engines are
    # shared, so putting these on the ACT DGE queue just interleaves their
    # packets with chunk 0's and delays row 0 by ~3us.
    fblob_sb = const.tile([128, 1 + 2 * RPC], F32, name="fblob_sb")
    nc.sync.dma_start(fblob_sb, t["fblob"].ap())
    bgn_sb = fblob_sb[:, 0:1]
    bias_sb = fblob_sb[:, 1 : 1 + 2 * RPC]

    # nbT split into kc halves so enb(kc0) lands/exps before chunk 1
    nbt_sb = const.tile([128, 2 * H * N], BF16, name="nbt_sb")
    nc.sync.dma_start(
        nbt_sb[:, 0 : H * N],
        bass.AP(t["nbT"], 0, [[2 * H * N, 128], [1, H * N]]),
    )
    nc.sync.dma_start(
        nbt_sb[:, H * N : 2 * H * N],
        bass.AP(t["nbT"], H * N, [[2 * H * N, 128], [1, H * N]]),
    )

    for i in range(1, len(CHUNK_ROWS)):
        load_chunk(i)

    if WITH_BO:
        bo_sb = const.tile([1, C], BF16, name="bo_sb")
        nc.sync.dma_start(bo_sb, t["bo_row"].ap())

    # one-time: exp(nonbatched_bias), k-major layout [p, kc*1024 + h*256 + q]
    # kc0 half exp'd immediately; kc1 half deferred into slot 1 (between
    # exp(0,0) and exp(0,1)) so it doesn't gate row 0's first exp.
    enb_sb = const.tile([128, 2 * H * N], BF16)
    nc.scalar.activation(enb_sb[:, 0:1024], nbt_sb[:, 0:1024], Act.Exp)

    def emit_enb1():
        nc.scalar.activation(
            enb_sb[:, 1024:2048], nbt_sb[:, 1024:2048], Act.Exp
        )

    # ---- PSUM: fixed 8-bank map ----
    ps = ctx.enter_context(tc.tile_pool(name="ps", bufs=1, space="PSUM"))
    lg_t = ps.tile([128, 4 * NB], F32, name="lg_t")  # B0-3: 4 x 256-col res
    wa_t = ps.tile([128, N], F32, name="wa_t", padded_shape=[128, NB])  # B4
    s_t = ps.tile([128, N], F32, name="s_t", padded_shape=[128, NB])  # B5
    px_t = ps.tile([128, 2 * NB], F32, name="px_t")  # B6-7 scratch

    # ---- SBUF working tiles ----
    sbq = ctx.enter_context(tc.tile_pool(name="sbq", bufs=4))
    sb3 = ctx.enter_context(tc.tile_pool(name="sb3", bufs=7))
    sbe = ctx.enter_context(tc.tile_pool(name="sbe", bufs=6))
    sbw = ctx.enter_context(tc.tile_pool(name="sbw", bufs=G // 2 + 5))
    nrow = RPC
    qk_sb = {}  # u -> [128, 4*N] bf16 (q_a | q_b | k_a | k_b)
    v_sb = [None] * nrow  # [128, 2*C] bf16
    e1_sb = {}  # u -> [128, 2*N] f32 gate exp for rows (2u, 2u+1)
    e_sb = {}  # (r, kc) -> [128, H*N] bf16
    wa_sb = {}  # u -> [128, 2*N] bf16 (rows 2u, 2u+1)
    wag_sb = {}  # u -> [128, 2*N] bf16
    d_bat = [
        const.tile([128, G * N], F32, name=f"d_bat{i}") for i in range(2)
    ]
    ln_bat = const.tile([128, G * N], F32, name="ln_bat")
    rs_bat = [
        const.tile([128, G * N], BF16, name=f"rs_bat{i}") for i in range(2)
    ]
    ostage = const.tile([128, 4 * N], BF16, name="ostage")  # 4 rows of out

    def xt_pair(r):
        ci, off = _CH_OF[r]
        return xt_ch[ci].rearrange(
            "p (r x) -> p r x", r=CHUNK_ROWS[ci] // 2
        )[:, off // 2, :]

    def mt_pair(r):
        ci, off = _CH_OF[r]
        return mt_ch[ci].rearrange(
            "p (r x) -> p r x", r=CHUNK_ROWS[ci] // 2
        )[:, off // 2, :]

    def mt_row(r):
        ci, off = _CH_OF[r]
        return mt_ch[ci].rearrange(
            "p (r x) -> p r x", r=CHUNK_ROWS[ci]
        )[:, off, :]

    out_dram = t["out"]

    # ---------------- per-stage emitters ----------------
    # superslot u: projections rows (2u, 2u+1); logits+exp rows (2u-2,
    # 2u-1); waU/S accumulation rows (2u-4, 2u-3).
    def st_wag(u):
        # wag = wa * rs for both rows of superslot u in one op, on GPSIMD
        # (issued as soon as rs is ready, never on the out-pop path).
        wag_sb[u] = sbw.tile([128, 2 * N], BF16, tag="wag", name=f"wag{u}")
        r0 = 2 * u
        nc.vector.tensor_mul(
            wag_sb[u],
            wa_sb[u],
            rs_bat[(r0 // G) % 2][:, (r0 % G) * N : (r0 % G) * N + 2 * N],
        )
        wa_sb[u] = None

    def st_out_mm(u):
        # c-major out pair: out[c, (rr,q)] = wo @ wag-pair, ONE 512-col MM
        # into a full px bank; bo added via a 1-partition broadcast matmul.
        bank = (u % 2) * NB
        MM(
            px_t[:, bank : bank + 2 * N],
            lhsT=wo_sb,
            rhs=wag_sb[u],
            start=True,
            stop=not WITH_BO,
        )
        if WITH_BO:
            MM(
                px_t[:, bank : bank + 2 * N],
                lhsT=bo_sb,
                rhs=ones512_sb,
                start=False,
                stop=True,
                skip_group_check=True,
            )
        wag_sb[u] = None

    def st_out_drain(u):
        # emitted separately so the ACT copy lands BETWEEN exps on the
        # scalar queue, filling the lg-bank WAR ring stall before each exp.
        # (Putting this on DVE instead was measured MUCH slower: the copy
        # lands deep in the DVE queue and its px read stalls the next
        # slot's matmuls.)
        bank = (u % 2) * NB
        half = ostage[:, (u % 2) * 2 * N : (u % 2) * 2 * N + 2 * N]
        nc.scalar.copy(half, px_t[:, bank : bank + 2 * N])
        dst = bass.AP(
            out_dram,
            u * 2 * C * N,
            [[N, 128], [C * N, 2], [1, N]],
        )
        nc.sync.dma_start(dst, half.rearrange("p (r x) -> p r x", r=2))

    def st_qk(u):
        # 2-row projections: q for rows (2u,2u+1) fills B6, k fills B7
        a = 2 * u
        MM(px_t[:, 0:NB], lhsT=wq_sb, rhs=xt_pair(a), start=True, stop=True)
        MM(px_t[:, NB : 2 * NB], lhsT=wk_sb, rhs=mt_pair(a), start=True, stop=True)
        qk_sb[u] = sbq.tile([128, 4 * N], BF16, tag="qk", name=f"qk{u}")
        nc.vector.tensor_copy(qk_sb[u], px_t)

    def st_v(r):
        MM(px_t[:, 0:C], lhsT=mt_row(r)[:, 0:128], rhs=wv_sb, start=True, stop=True)
        MM(
            px_t[:, NB : NB + C],
            lhsT=mt_row(r)[:, 128:256],
            rhs=wv_sb,
            start=True,
            stop=True,
        )
        v_sb[r] = sb3.tile([128, 2 * C], BF16, tag="v", name=f"v{r}")
        nc.vector.tensor_copy(
            v_sb[r].rearrange("p (b x) -> p b x", b=2),
            px_t.rearrange("p (b x) -> p b x", b=2)[:, :, 0:C],
        )

    def st_g(u):
        # 2-row gate projection fills B6 [g_a | g_b]
        MM(px_t[:, 0:NB], lhsT=wg_sb, rhs=xt_pair(2 * u), start=True, stop=True)

    def st_e1(u):
        # e1 = exp(-(gpre+bg)) for both rows, one ACT call from PSUM
        e1_sb[u] = sb3.tile([128, 2 * N], F32, tag="ge1", name=f"ge1_{u}")
        nc.scalar.activation(
            e1_sb[u], px_t[:, 0:NB], Act.Exp, bias=bgn_sb, scale=-1.0
        )

    def st_lg(r, kc):
        # logitsT[ktok, q] = k_h @ q_h.T, 4 heads row-tiled -> B0..B3.
        # (PSUM matmul outputs must be bank-aligned on this build — a
        # half-bank col-offset layout hard-faults at runtime — so lg is
        # single-buffered in banks 0-3 and lg(r,kc') serializes on
        # exp(r,kc) having drained them.)
        qk_t = qk_sb[r // 2]
        qoff = (r % 2) * N
        koff = 2 * N + (r % 2) * N
        for h in range(H):
            MM(
                lg_t[:, NB * h : NB * h + N],
                lhsT=qk_t[
                    32 * h : 32 * h + 32, koff + 128 * kc : koff + 128 * kc + 128
                ],
                rhs=qk_t[32 * h : 32 * h + 32, qoff : qoff + N],
                start=True,
                stop=True,
                tile_position=(32 * h, 0),
            )
        # one exp for all 4 heads; bias col per (kc, r); then *exp(nb)
        et = sbe.tile([128, H * N], BF16, tag=f"e{kc}", name=f"e{kc}_{r}")
        e_sb[(r, kc)] = et
        nc.scalar.activation(
            et.rearrange("p (b x) -> p b x", b=4),
            lg_t.rearrange("p (b x) -> p b x", b=4)[:, :, 0:N],
            Act.Exp,
            bias=bias_sb[:, kc * RPC + r : kc * RPC + r + 1],
            scale=KEY_SCALE,
        )
        nc.vector.tensor_mul(
            et,
            et,
            enb_sb[:, 1024 * kc : 1024 * kc + 1024],
        )

    def et_head(et, h):
        off = h * N
        return et[:, off : off + N]

    def st_waS(r, kc):
        # waU += v_h.T @ e_h (col-tiled by head into B4);  S += 1.T @ e_h (B5)
        et = e_sb[(r, kc)]
        for h in range(H):
            MM(
                wa_t[32 * h : 32 * h + 32, :],
                lhsT=v_sb[r][:, 128 * kc + 32 * h : 128 * kc + 32 * h + 32],
                rhs=et_head(et, h),
                start=(kc == 0),
                stop=(kc == 1),
                tile_position=(0, 32 * h),
                skip_group_check=True,
            )
        for h in range(H):
            MM(
                s_t[32 * h : 32 * h + 32, :],
                lhsT=ones32_sb,
                rhs=et_head(et, h),
                start=(kc == 0),
                stop=(kc == 1),
                tile_position=(0, 32 * h),
                skip_group_check=True,
            )
        e_sb[(r, kc)] = None
        if kc == 1:
            v_sb[r] = None
            # d = (1+e1) * S first (it feeds the batched ln on ACT — the
            # latency-critical path), then drain waU into the pair tile.
            u = r // 2
            nc.vector.scalar_tensor_tensor(
                d_bat[(r // G) % 2][:, (r % G) * N : (r % G) * N + N],
                e1_sb[u][:, (r % 2) * N : (r % 2) * N + N],
                1.0,
                s_t,
                mybir.AluOpType.add,
                mybir.AluOpType.mult,
            )
            if r % 2 == 0:
                wa_sb[u] = sbw.tile(
                    [128, 2 * N], BF16, tag="wa", name=f"wa{u}"
                )
            nc.vector.tensor_copy(
                wa_sb[u][:, (r % 2) * N : (r % 2) * N + N], wa_t
            )

    def st_lnrs(rlast, n=G):
        # batched ln + reciprocal-exp for rows [rlast-n+1, rlast]
        base = ((rlast - n + 1) % G) * N
        width = n * N
        buf = (rlast // G) % 2
        nc.scalar.activation(
            ln_bat[:, base : base + width],
            d_bat[buf][:, base : base + width],
            Act.Ln,
        )
        nc.scalar.activation(
            rs_bat[buf][:, base : base + width],
            ln_bat[:, base : base + width],
            Act.Exp,
            scale=-1.0,
        )

    # PE warmup burst on memset tiles (no DMA dependency): ramps the HAM
    # clock gate while the framework preamble + input DMAs run.
    for w in range(8):
        MM(
            lg_t[0:DH, (w % 4) * NB : (w % 4) * NB + N],
            lhsT=ones32_sb,
            rhs=warm_sb,
            start=True,
            stop=True,
        )

    # ---------------- the software-pipelined superslot loop ----------------
    # slot u: projections rows (2u, 2u+1); logits+exp rows lp=(2u-2, 2u-1);
    # waU/S rows (2u-4, 2u-3).  B4/B5 hold ONE open accumulation group:
    #   slot u: close(2u-5), open(2u-4), close(2u-4), open(2u-3)
    NU = RPC // 2
    out_q = []  # superslots with wag ready, waiting for out stage
    for u in range(NU + 4 + G // 2 + 2):
        a, b = 2 * u, 2 * u + 1  # projection rows this slot
        la, lb = 2 * u - 2, 2 * u - 1  # logits+exp rows
        wp, wq_ = 2 * u - 4, 2 * u - 3  # waU/S rows
        batch_ends = []

        def close_row(r):
            st_waS(r, 1)
            # last G-batch split into two 2-row halves so the final out
            # pops start a slot earlier (shorter drain tail)
            if r in (RPC - 3, RPC - 1):
                batch_ends.append((r, 2))
            elif r % G == G - 1:
                batch_ends.append((r, G))

        pops = []
        while out_q and len(pops) < 2:
            up = out_q.pop(0)
            st_out_mm(up)
            pops.append(up)
        if 0 <= la < RPC:
            st_lg(la, 0)
        if u == 1:
            emit_enb1()  # deferred kc1 exp(nb): after exp(0,0) on ACT
        if pops:
            st_out_drain(pops[0])  # ACT filler between exp(la,0)/(la,1)
        if 0 <= wp - 1 < RPC and u >= 1:
            close_row(wp - 1)  # row 2u-5
        if 0 <= la < RPC:
            st_lg(la, 1)
        if len(pops) > 1:
            st_out_drain(pops[1])  # ACT filler between exp(la,1)/(lb,0)
        if u < NU:
            st_qk(u)
        if 0 <= wp < RPC:
            st_waS(wp, 0)
        if 0 <= lb < RPC:
            st_lg(lb, 0)
        if u == 0:
            st_g(u)
            st_e1(u)
        if u < NU:
            st_v(a)
            st_v(b)
        if 0 <= wp < RPC:
            close_row(wp)  # row 2u-4
        if 0 < u < NU:
            st_g(u)
            st_e1(u)  # ACT filler between exp(lb,0)/(lb,1)
        if 0 <= lb < RPC:
            st_lg(lb, 1)
        if 0 <= wq_ < RPC:
            st_waS(wq_, 0)
        for be, bn in batch_ends:
            st_lnrs(be, bn)  # ACT filler into next slot's exp(la,0)
            for up in range((be - bn + 1) // 2, (be + 1) // 2):
                st_wag(up)
                out_q.append(up)


def _build():
    if "nc" in _CACHE:
        return _CACHE["nc"], _CACHE["t"]
    nc = bass.Bass(
        "TRN2", target_bir_lowering=False, debug=False, num_devices=NCORES
    )
    t = {}
    t["xt"] = nc.dram_tensor("xt", [RPC, C, N], BF16, kind="ExternalInput")
    t["mt"] = nc.dram_tensor("mt", [RPC, C, N], BF16, kind="ExternalInput")
    t["wblob"] = nc.dram_tensor("wblob", [128, 5 * C], BF16, kind="ExternalInput")
    t["fblob"] = nc.dram_tensor("fblob", [128, 1 + 2 * RPC], F32, kind="ExternalInput")
    t["nbT"] = nc.dram_tensor("nbT", [128, 2 * H * N], BF16, kind="ExternalInput")
    if WITH_BO:
        t["bo_row"] = nc.dram_tensor("bo_row", [1, C], BF16, kind="ExternalInput")
    t["out"] = nc.dram_tensor("out", [RPC, C, N], BF16, kind="ExternalOutput")

    with tile.TileContext(nc) as tc:
        with ExitStack() as ctx:
            _emit(ctx, tc, t)
    _legalize_multiwaits(nc, max_waits=1)
    _CACHE["nc"] = nc
    _CACHE["t"] = t
    return nc, t


def _prep_in_maps(q_data, m_data, bias, nonbatched_bias, wq, wk, wv, wo, bo, wg, bg):
    bf16 = mybir.dt.np(BF16)
    q_data = np.ascontiguousarray(np.asarray(q_data, np.float32))
    m_data = np.ascontiguousarray(np.asarray(m_data, np.float32))
    bias = np.asarray(bias, np.float32)
    nb = np.asarray(nonbatched_bias, np.float32)

    # pure layout prep (transposes/reshapes); all math stays on device
    wblob = np.concatenate(
        [
            np.asarray(w, np.float32).T.astype(bf16)
            for w in (wq, wk, wv, wg, wo)
        ],
        axis=1,
    )
    consts = {
        "wblob": np.ascontiguousarray(wblob),
        # nbT[p, kc*1024 + j*256 + q] = nb[0, h, q, kc*128+p] with the
        # bank-major head order j: [h0, h2, h1, h3] (matches et layout)
        "nbT": np.ascontiguousarray(
            nb[0]
            .transpose(2, 0, 1)  # [k, h, q]
            .reshape(2, 128, H, N)
            .transpose(1, 0, 2, 3)
            .reshape(128, 2 * H * N)
            .astype(bf16)
        ),
    }
    if WITH_BO:
        consts["bo_row"] = np.ascontiguousarray(
            np.asarray(bo, np.float32)[None, :].astype(bf16)
        )
    bgn_col = (-np.asarray(bg, np.float32))[:, None]
    # bias_r[p, kc*RPC + r] = bias[0, n0+r, 0, 0, kc*128+p]
    bias_kn = bias[0, :, 0, 0, :].T.reshape(2, 128, N)  # [kc, p, n]
    in_maps = []
    for c in range(NCORES):
        n0 = c * RPC
        rows = slice(n0, n0 + RPC)
        m = dict(consts)
        m["xt"] = np.ascontiguousarray(q_data[0, rows].transpose(0, 2, 1).astype(bf16))
        m["mt"] = np.ascontiguousarray(m_data[0, rows].transpose(0, 2, 1).astype(bf16))
        bias_r = bias_kn[:, :, rows].transpose(1, 0, 2).reshape(128, 2 * RPC)
        m["fblob"] = np.ascontiguousarray(
            np.concatenate([bgn_col, bias_r], axis=1, dtype=np.float32)
        )
        in_maps.append(m)
    return in_maps


def kernel(**inputs) -> np.ndarray:
    global WITH_BO
    want_bo = bool(np.any(np.asarray(inputs["bo"]) != 0))
    if want_bo != WITH_BO or "nc" not in _CACHE:
        WITH_BO = want_bo
        _CACHE.clear()
    nc, _ = _build()
    in_maps = _prep_in_maps(**inputs)
    res = run_bass_kernel_spmd(nc, in_maps, core_ids=list(range(NCORES)))
    out = np.concatenate(
        [
            res.results[c]["out"].astype(np.float32).transpose(0, 2, 1)
            for c in range(NCORES)
        ],
        axis=0,
    )
    return np.ascontiguousarray(out.reshape(B, N, N, C).astype(np.float32))


if __name__ == "__main__":
    # smoke test against a tiny numpy reference
    rng = np.random.default_rng(0)
    inputs = {
        "q_data": rng.standard_normal((B, N, N, C)).astype(np.float32),
        "m_data": rng.standard_normal((B, N, N, C)).astype(np.float32),
        "bias": rng.standard_normal((B, N, 1, 1, N)).astype(np.float32),
        "nonbatched_bias": rng.standard_normal((1, H, N, N)).astype(np.float32),
        "wq": (rng.standard_normal((C, C)) / np.sqrt(C)).astype(np.float32),
        "wk": (rng.standard_normal((C, C)) / np.sqrt(C)).astype(np.float32),
        "wv": (rng.standard_normal((C, C)) / np.sqrt(C)).astype(np.float32),
        "wo": (rng.standard_normal((C, C)) / np.sqrt(C)).astype(np.float32),
        "bo": np.zeros((C,), np.float32),
        "wg": np.ones((C, C), np.float32) / np.sqrt(C),
        "bg": np.ones((C,), np.float32),
    }
    out = kernel(**inputs)
    print("out", out.shape, out.dtype, float(np.abs(out).max()))


# revision 55
# speedup vs baseline: 1.2103x; 1.0064x over previous
"""Trainium2 Bass kernel for gated multi-head pair attention (AlphaFold-style).

Reference computation (B=1, N=256, C=128, H=4, DH=32):
    q = (q_data @ wq.T) * DH**-0.5        # [B,N,Nq,C]
    k = m_data @ wk.T ; v = m_data @ wv.T
    logits = einsum("bnqhd,bnkhd->bnhqk", q, k) + bias + nonbatched_bias
    weight = softmax(logits, axis=-1)
    wa = einsum("bnhqk,bnkhd->bnqhd", weight, v)
    g  = sigmoid(q_data @ wg.T + bg)
    out = (wa * g).reshape(...) @ wo.T + bo

Sharding: pure data-parallel across the 8 NeuronCores along the first
residue axis (N): core c owns rows [32c, 32c+32). Params + nonbatched_bias
replicated.

v4 (four-engine pipeline): k-major like v2/v3, restructured around the
measured per-engine workload (ACT was the binding engine at ~3.45us/row):
  - four-engine split: ACT keeps only exp/ln work (et-exps, gate-exp,
    batched ln+rs).  GPSIMD (idle before) takes the kc0 et*exp(nb)
    multiply and the wag=wa*rs multiply (SBUF-only tensor ops DO compile
    on this build; PSUM access does not).  DVE keeps the PSUM drains, the
    kc1 multiply, d=(1+e1)*S, and the out staging copy.
  - deeper software pipeline: logits+exp for rows (2u-2, 2u-1) and
    waU/S accumulation for rows (2u-4, 2u-3) run in superslot u, so the
    slow GPSIMD multiply (~2.1us) is off the PE's in-order critical path.
  - wag is computed at rs-batch completion time (not at out-pop time) so
    the out matmul never waits on GPSIMD latency.
  - DMA: consts packed into two blobs; input DMAs split across the SP and
    ACT hardware DGE queues (fblob+nbT on ACT, xt/mt chunks on SP) so the
    nonbatched-bias transfer doesn't serialize behind the input chunks;
    first chunk is only 2 rows so row 0 can start ~4us earlier.
  - exp(nb) is loaded bf16 and exp'd once; bias folded into the et-exp's
    per-partition bias port; softmax denom and sigmoid gate fused into
    ONE reciprocal chain rs = exp(-ln((1+e1)*S)).

Environment notes (this walrus build): one sem wait max per instruction
(_legalize_multiwaits); two matmuls must never concurrently target
different column ranges of the same PSUM bank; gpsimd cannot access PSUM;
no PSUM-source DMAs; only exp/ln ACT funcs are used so the ACT table set
loads exactly once.  Measured: the 4 tile-positioned matmuls of a group
stream their output columns serially (cost = total cols at 2.4GHz);
lg-bank kc double-buffering is SLOWER (PSUM port contention with the
exp's reads); gpsimd Multiply eff=0.42 -> [128,1024] mul ~2.1us.
"""

import os
import sys

sys.path.insert(0, "/opt/trn_rl_repo")

from contextlib import ExitStack

import numpy as np

import concourse.bass as bass
import concourse.tile as tile
from concourse import mybir
from concourse.bass_utils import run_bass_kernel_spmd

B, N, C, H = 1, 256, 128, 4
DH = C // H
KEY_SCALE = DH**-0.5
NCORES = 8
RPC = N // NCORES  # rows per core
G = 4  # rows per batched ln/rs epilogue call
CHUNK_ROWS = [2, 6, 8, 8, 8]  # uneven input chunks: row 0 starts early

F32 = mybir.dt.float32
BF16 = mybir.dt.bfloat16

WITH_BO = True  # set by kernel() per-input; bo==0 skips the bias matmuls

_CACHE = {}


def _legalize_multiwaits(nc, max_waits=1):
    """The walrus build here encodes at most one sem wait per instruction
    ("Too many sync wait commands" otherwise). Split excess waits onto
    freshly inserted Drain instructions on the same engine just before the
    multi-wait instruction (engines execute in order, so this is
    equivalent)."""
    n_fix = 0
    for f in nc.m.functions:
        for blk in f.blocks:
            changed = False
            new_insts = []
            for inst in blk.instructions:
                si = inst.sync_info
                ow = list(si.on_wait) if (si is not None and si.on_wait) else []
                if len(ow) > max_waits:
                    head, tail = ow[:-max_waits], ow[-max_waits:]
                    while head:
                        chunk, head = head[:max_waits], head[max_waits:]
                        d = mybir.InstNoOp(
                            name=f"I-mw{nc.next_id()}", ins=[], outs=[]
                        )
                        d.engine = inst.engine
                        d.sync_info = mybir.SyncInfo(
                            on_wait=list(chunk), on_update=[]
                        )
                        new_insts.append(d)
                        n_fix += 1
                    inst.sync_info = mybir.SyncInfo(
                        on_wait=list(tail),
                        on_update=list(si.on_update) if si.on_update else [],
                    )
                    changed = True
                new_insts.append(inst)
            if changed:
                blk.instructions = new_insts
    return n_fix


# row -> (chunk index, row offset inside chunk)
_CH_OF = []
for _ci, _n in enumerate(CHUNK_ROWS):
    for _j in range(_n):
        _CH_OF.append((_ci, _j))
_CH_BASE = [sum(CHUNK_ROWS[:i]) for i in range(len(CHUNK_ROWS))]


def _emit(ctx: ExitStack, tc: "tile.TileContext", t):
    nc = tc.nc
    MM = nc.tensor.matmul
    Act = mybir.ActivationFunctionType
    NB = 512  # psum bank stride (fp32 elems)

    const = ctx.enter_context(tc.tile_pool(name="const", bufs=1))

    # warmup rhs + ones tiles first: memset-only (no DMA dependency) so the
    # PE ramp burst can start immediately.
    warm_sb = const.tile([128, N], BF16, name="warm_sb")
    nc.vector.memset(warm_sb, 0.0)
    ones32_sb = const.tile([128, DH], BF16)
    nc.vector.memset(ones32_sb, 1.0)
    if WITH_BO:
        ones512_sb = const.tile([1, 2 * N], BF16)
        nc.vector.memset(ones512_sb, 1.0)

    # consolidated const DMAs, ordered for the row-0 critical path on the
    # SP queue: wblob -> xt/mt chunk 0 -> remaining chunks.  fblob + nbT
    # go down the ACT engine's independent HW DGE queue in parallel.
    wblob_sb = const.tile([128, 5 * C], BF16, name="wblob_sb")
    nc.sync.dma_start(wblob_sb, t["wblob"].ap())
    wq_sb = wblob_sb[:, 0 * C : 1 * C]
    wk_sb = wblob_sb[:, 1 * C : 2 * C]
    wv_sb = wblob_sb[:, 2 * C : 3 * C]
    wg_sb = wblob_sb[:, 3 * C : 4 * C]
    wo_sb = wblob_sb[:, 4 * C : 5 * C]

    xt_ch = [
        const.tile([128, n * N], BF16, name=f"xt_ch{i}")
        for i, n in enumerate(CHUNK_ROWS)
    ]
    mt_ch = [
        const.tile([128, n * N], BF16, name=f"mt_ch{i}")
        for i, n in enumerate(CHUNK_ROWS)
    ]

    def load_chunk(i):
        nrows = CHUNK_ROWS[i]
        for dram, sbuf in ((t["xt"], xt_ch[i]), (t["mt"], mt_ch[i])):
            srcap = bass.AP(
                dram,
                _CH_BASE[i] * C * N,
                [[N, 128], [C * N, nrows], [1, N]],
            )
            nc.sync.dma_start(
                sbuf.rearrange("p (r x) -> p r x", r=nrows), srcap
            )

    load_chunk(0)

    # fblob + nbT after chunk 0 on the same SP queue: the DMA engines are
    # shared, so putting these on the ACT DGE queue just interleaves their
    # packets with chunk 0's and delays row 0 by ~3us.
    fblob_sb = const.tile([128, 1 + 2 * RPC], F32, name="fblob_sb")
    nc.sync.dma_start(fblob_sb, t["fblob"].ap())
    bgn_sb = fblob_sb[:, 0:1]
    bias_sb = fblob_sb[:, 1 : 1 + 2 * RPC]

    # nbT split into kc halves so enb(kc0) lands/exps before chunk 1
    nbt_sb = const.tile([128, 2 * H * N], BF16, name="nbt_sb")
    nc.sync.dma_start(
        nbt_sb[:, 0 : H * N],
        bass.AP(t["nbT"], 0, [[2 * H * N, 128], [1, H * N]]),
    )
    nc.sync.dma_start(
        nbt_sb[:, H * N : 2 * H * N],
        bass.AP(t["nbT"], H * N, [[2 * H * N, 128], [1, H * N]]),
    )

    for i in range(1, len(CHUNK_ROWS)):
        load_chunk(i)

    if WITH_BO:
        bo_sb = const.tile([1, C], BF16, name="bo_sb")
        nc.sync.dma_start(bo_sb, t["bo_row"].ap())

    # one-time: exp(nonbatched_bias), k-major layout [p, kc*1024 + h*256 + q]
    # kc0 half exp'd immediately; kc1 half deferred into slot 1 (between
    # exp(0,0) and exp(0,1)) so it doesn't gate row 0's first exp.
    enb_sb = const.tile([128, 2 * H * N], BF16)
    nc.scalar.activation(enb_sb[:, 0:1024], nbt_sb[:, 0:1024], Act.Exp)

    def emit_enb1():
        nc.scalar.activation(
            enb_sb[:, 1024:2048], nbt_sb[:, 1024:2048], Act.Exp
        )

    # ---- PSUM: fixed 8-bank map ----
    ps = ctx.enter_context(tc.tile_pool(name="ps", bufs=1, space="PSUM"))
    lg_t = ps.tile([128, 4 * NB], F32, name="lg_t")  # B0-3: 4 x 256-col res
    wa_t = ps.tile([128, N], F32, name="wa_t", padded_shape=[128, NB])  # B4
    s_t = ps.tile([128, N], F32, name="s_t", padded_shape=[128, NB])  # B5
    px_t = ps.tile([128, 2 * NB], F32, name="px_t")  # B6-7 scratch

    # ---- SBUF working tiles ----
    sbq = ctx.enter_context(tc.tile_pool(name="sbq", bufs=4))
    sb3 = ctx.enter_context(tc.tile_pool(name="sb3", bufs=7))
    sbe = ctx.enter_context(tc.tile_pool(name="sbe", bufs=6))
    sbw = ctx.enter_context(tc.tile_pool(name="sbw", bufs=G // 2 + 5))
    nrow = RPC
    qk_sb = {}  # u -> [128, 4*N] bf16 (q_a | q_b | k_a | k_b)
    v_sb = [None] * nrow  # [128, 2*C] bf16
    e1_sb = {}  # u -> [128, 2*N] f32 gate exp for rows (2u, 2u+1)
    e_sb = {}  # (r, kc) -> [128, H*N] bf16
    wa_sb = {}  # u -> [128, 2*N] bf16 (rows 2u, 2u+1)
    wag_sb = {}  # u -> [128, 2*N] bf16
    d_bat = [
        const.tile([128, G * N], F32, name=f"d_bat{i}") for i in range(2)
    ]
    ln_bat = const.tile([128, G * N], F32, name="ln_bat")
    rs_bat = [
        const.tile([128, G * N], BF16, name=f"rs_bat{i}") for i in range(2)
    ]
    ostage = const.tile([128, 4 * N], BF16, name="ostage")  # 4 rows of out

    def xt_pair(r):
        ci, off = _CH_OF[r]
        return xt_ch[ci].rearrange(
            "p (r x) -> p r x", r=CHUNK_ROWS[ci] // 2
        )[:, off // 2, :]

    def mt_pair(r):
        ci, off = _CH_OF[r]
        return mt_ch[ci].rearrange(
            "p (r x) -> p r x", r=CHUNK_ROWS[ci] // 2
        )[:, off // 2, :]

    def mt_row(r):
        ci, off = _CH_OF[r]
        return mt_ch[ci].rearrange(
            "p (r x) -> p r x", r=CHUNK_ROWS[ci]
        )[:, off, :]

    out_dram = t["out"]

    # ---------------- per-stage emitters ----------------
    # superslot u: projections rows (2u, 2u+1); logits+exp rows (2u-2,
    # 2u-1); waU/S accumulation rows (2u-4, 2u-3).
    def st_wag(u):
        # wag = wa * rs for both rows of superslot u in one op, on GPSIMD
        # (issued as soon as rs is ready, never on the out-pop path).
        wag_sb[u] = sbw.tile([128, 2 * N], BF16, tag="wag", name=f"wag{u}")
        r0 = 2 * u
        nc.vector.tensor_mul(
            wag_sb[u],
            wa_sb[u],
            rs_bat[(r0 // G) % 2][:, (r0 % G) * N : (r0 % G) * N + 2 * N],
        )
        wa_sb[u] = None

    def st_out_mm(u):
        # c-major out pair: out[c, (rr,q)] = wo @ wag-pair, ONE 512-col MM
        # into a full px bank; bo added via a 1-partition broadcast matmul.
        bank = (u % 2) * NB
        MM(
            px_t[:, bank : bank + 2 * N],
            lhsT=wo_sb,
            rhs=wag_sb[u],
            start=True,
            stop=not WITH_BO,
        )
        if WITH_BO:
            MM(
                px_t[:, bank : bank + 2 * N],
                lhsT=bo_sb,
                rhs=ones512_sb,
                start=False,
                stop=True,
                skip_group_check=True,
            )
        wag_sb[u] = None

    def st_out_drain(u):
        # emitted separately so the ACT copy lands BETWEEN exps on the
        # scalar queue, filling the lg-bank WAR ring stall before each exp.
        # (Putting this on DVE instead was measured MUCH slower: the copy
        # lands deep in the DVE queue and its px read stalls the next
        # slot's matmuls.)
        bank = (u % 2) * NB
        half = ostage[:, (u % 2) * 2 * N : (u % 2) * 2 * N + 2 * N]
        nc.scalar.copy(half, px_t[:, bank : bank + 2 * N])
        dst = bass.AP(
            out_dram,
            u * 2 * C * N,
            [[N, 128], [C * N, 2], [1, N]],
        )
        nc.sync.dma_start(dst, half.rearrange("p (r x) -> p r x", r=2))

    def st_qk(u):
        # 2-row projections: q for rows (2u,2u+1) fills B6, k fills B7
        a = 2 * u
        MM(px_t[:, 0:NB], lhsT=wq_sb, rhs=xt_pair(a), start=True, stop=True)
        MM(px_t[:, NB : 2 * NB], lhsT=wk_sb, rhs=mt_pair(a), start=True, stop=True)
        qk_sb[u] = sbq.tile([128, 4 * N], BF16, tag="qk", name=f"qk{u}")
        nc.vector.tensor_copy(qk_sb[u], px_t)

    def st_v(r):
        MM(px_t[:, 0:C], lhsT=mt_row(r)[:, 0:128], rhs=wv_sb, start=True, stop=True)
        MM(
            px_t[:, NB : NB + C],
            lhsT=mt_row(r)[:, 128:256],
            rhs=wv_sb,
            start=True,
            stop=True,
        )
        v_sb[r] = sb3.tile([128, 2 * C], BF16, tag="v", name=f"v{r}")
        nc.vector.tensor_copy(
            v_sb[r].rearrange("p (b x) -> p b x", b=2),
            px_t.rearrange("p (b x) -> p b x", b=2)[:, :, 0:C],
        )

    def st_g(u):
        # 2-row gate projection fills B6 [g_a | g_b]
        MM(px_t[:, 0:NB], lhsT=wg_sb, rhs=xt_pair(2 * u), start=True, stop=True)

    def st_e1(u):
        # e1 = exp(-(gpre+bg)) for both rows, one ACT call from PSUM
        e1_sb[u] = sb3.tile([128, 2 * N], F32, tag="ge1", name=f"ge1_{u}")
        nc.scalar.activation(
            e1_sb[u], px_t[:, 0:NB], Act.Exp, bias=bgn_sb, scale=-1.0
        )

    def st_lg(r, kc):
        # logitsT[ktok, q] = k_h @ q_h.T, 4 heads row-tiled -> B0..B3.
        # (PSUM matmul outputs must be bank-aligned on this build — a
        # half-bank col-offset layout hard-faults at runtime — so lg is
        # single-buffered in banks 0-3 and lg(r,kc') serializes on
        # exp(r,kc) having drained them.)
        qk_t = qk_sb[r // 2]
        qoff = (r % 2) * N
        koff = 2 * N + (r % 2) * N
        for h in range(H):
            MM(
                lg_t[:, NB * h : NB * h + N],
                lhsT=qk_t[
                    32 * h : 32 * h + 32, koff + 128 * kc : koff + 128 * kc + 128
                ],
                rhs=qk_t[32 * h : 32 * h + 32, qoff : qoff + N],
                start=True,
                stop=True,
                tile_position=(32 * h, 0),
            )
        # one exp for all 4 heads; bias col per (kc, r); then *exp(nb)
        et = sbe.tile([128, H * N], BF16, tag=f"e{kc}", name=f"e{kc}_{r}")
        e_sb[(r, kc)] = et
        nc.scalar.activation(
            et.rearrange("p (b x) -> p b x", b=4),
            lg_t.rearrange("p (b x) -> p b x", b=4)[:, :, 0:N],
            Act.Exp,
            bias=bias_sb[:, kc * RPC + r : kc * RPC + r + 1],
            scale=KEY_SCALE,
        )
        nc.vector.tensor_mul(
            et,
            et,
            enb_sb[:, 1024 * kc : 1024 * kc + 1024],
        )

    def et_head(et, h):
        off = h * N
        return et[:, off : off + N]

    def st_waS(r, kc):
        # waU += v_h.T @ e_h (col-tiled by head into B4);  S += 1.T @ e_h (B5)
        et = e_sb[(r, kc)]
        for h in range(H):
            MM(
                wa_t[32 * h : 32 * h + 32, :],
                lhsT=v_sb[r][:, 128 * kc + 32 * h : 128 * kc + 32 * h + 32],
                rhs=et_head(et, h),
                start=(kc == 0),
                stop=(kc == 1),
                tile_position=(0, 32 * h),
                skip_group_check=True,
            )
        for h in range(H):
            MM(
                s_t[32 * h : 32 * h + 32, :],
                lhsT=ones32_sb,
                rhs=et_head(et, h),
                start=(kc == 0),
                stop=(kc == 1),
                tile_position=(0, 32 * h),
                skip_group_check=True,
            )
        e_sb[(r, kc)] = None
        if kc == 1:
            v_sb[r] = None
            # d = (1+e1) * S first (it feeds the batched ln on ACT — the
            # latency-critical path), then drain waU into the pair tile.
            u = r // 2
            nc.vector.scalar_tensor_tensor(
                d_bat[(r // G) % 2][:, (r % G) * N : (r % G) * N + N],
                e1_sb[u][:, (r % 2) * N : (r % 2) * N + N],
                1.0,
                s_t,
                mybir.AluOpType.add,
                mybir.AluOpType.mult,
            )
            if r % 2 == 0:
                wa_sb[u] = sbw.tile(
                    [128, 2 * N], BF16, tag="wa", name=f"wa{u}"
                )
            nc.vector.tensor_copy(
                wa_sb[u][:, (r % 2) * N : (r % 2) * N + N], wa_t
            )

    def st_lnrs(rlast, n=G):
        # batched ln + reciprocal-exp for rows [rlast-n+1, rlast]
        base = ((rlast - n + 1) % G) * N
        width = n * N
        buf = (rlast // G) % 2
        nc.scalar.activation(
            ln_bat[:, base : base + width],
            d_bat[buf][:, base : base + width],
            Act.Ln,
        )
        nc.scalar.activation(
            rs_bat[buf][:, base : base + width],
            ln_bat[:, base : base + width],
            Act.Exp,
            scale=-1.0,
        )

    # PE warmup burst on memset tiles (no DMA dependency): ramps the HAM
    # clock gate while the framework preamble + input DMAs run.
    for w in range(16):
        MM(
            lg_t[0:DH, (w % 4) * NB : (w % 4) * NB + N],
            lhsT=ones32_sb,
            rhs=warm_sb,
            start=True,
            stop=True,
        )

    # ---------------- the software-pipelined superslot loop ----------------
    # slot u: projections rows (2u, 2u+1); logits+exp rows lp=(2u-2, 2u-1);
    # waU/S rows (2u-4, 2u-3).  B4/B5 hold ONE open accumulation group:
    #   slot u: close(2u-5), open(2u-4), close(2u-4), open(2u-3)
    NU = RPC // 2
    out_q = []  # superslots with wag ready, waiting for out stage
    for u in range(NU + 4 + G // 2 + 2):
        a, b = 2 * u, 2 * u + 1  # projection rows this slot
        la, lb = 2 * u - 2, 2 * u - 1  # logits+exp rows
        wp, wq_ = 2 * u - 4, 2 * u - 3  # waU/S rows
        batch_ends = []

        def close_row(r):
            st_waS(r, 1)
            # last G-batch split into two 2-row halves so the final out
            # pops start a slot earlier (shorter drain tail)
            if r in (RPC - 3, RPC - 1):
                batch_ends.append((r, 2))
            elif r % G == G - 1:
                batch_ends.append((r, G))

        pops = []
        while out_q and len(pops) < 2:
            up = out_q.pop(0)
            st_out_mm(up)
            pops.append(up)
        if 0 <= la < RPC:
            st_lg(la, 0)
        if u == 1:
            emit_enb1()  # deferred kc1 exp(nb): after exp(0,0) on ACT
        if pops:
            st_out_drain(pops[0])  # ACT filler between exp(la,0)/(la,1)
        if 0 <= wp - 1 < RPC and u >= 1:
            close_row(wp - 1)  # row 2u-5
        if 0 <= la < RPC:
            st_lg(la, 1)
        if len(pops) > 1:
            st_out_drain(pops[1])  # ACT filler between exp(la,1)/(lb,0)
        if u < NU:
            st_qk(u)
        if 0 <= wp < RPC:
            st_waS(wp, 0)
        if 0 <= lb < RPC:
            st_lg(lb, 0)
        if u == 0:
            st_g(u)
            st_e1(u)
        if u < NU:
            st_v(a)
            st_v(b)
        if 0 <= wp < RPC:
            close_row(wp)  # row 2u-4
        if 0 < u < NU:
            st_g(u)
            st_e1(u)  # ACT filler between exp(lb,0)/(lb,1)
        if 0 <= lb < RPC:
            st_lg(lb, 1)
        if 0 <= wq_ < RPC:
            st_waS(wq_, 0)
        for be, bn in batch_ends:
            st_lnrs(be, bn)  # ACT filler into next slot's exp(la,0)
            for up in range((be - bn + 1) // 2, (be + 1) // 2):
                st_wag(up)
                out_q.append(up)


def _build():
    if "nc" in _CACHE:
        return _CACHE["nc"], _CACHE["t"]
    nc = bass.Bass(
        "TRN2", target_bir_lowering=False, debug=False, num_devices=NCORES
    )
    t = {}
    t["xt"] = nc.dram_tensor("xt", [RPC, C, N], BF16, kind="ExternalInput")
    t["mt"] = nc.dram_tensor("mt", [RPC, C, N], BF16, kind="ExternalInput")
    t["wblob"] = nc.dram_tensor("wblob", [128, 5 * C], BF16, kind="ExternalInput")
    t["fblob"] = nc.dram_tensor("fblob", [128, 1 + 2 * RPC], F32, kind="ExternalInput")
    t["nbT"] = nc.dram_tensor("nbT", [128, 2 * H * N], BF16, kind="ExternalInput")
    if WITH_BO:
        t["bo_row"] = nc.dram_tensor("bo_row", [1, C], BF16, kind="ExternalInput")
    t["out"] = nc.dram_tensor("out", [RPC, C, N], BF16, kind="ExternalOutput")

    with tile.TileContext(nc) as tc:
        with ExitStack() as ctx:
            _emit(ctx, tc, t)
    _legalize_multiwaits(nc, max_waits=1)
    _CACHE["nc"] = nc
    _CACHE["t"] = t
    return nc, t


def _prep_in_maps(q_data, m_data, bias, nonbatched_bias, wq, wk, wv, wo, bo, wg, bg):
    bf16 = mybir.dt.np(BF16)
    q_data = np.ascontiguousarray(np.asarray(q_data, np.float32))
    m_data = np.ascontiguousarray(np.asarray(m_data, np.float32))
    bias = np.asarray(bias, np.float32)
    nb = np.asarray(nonbatched_bias, np.float32)

    # pure layout prep (transposes/reshapes); all math stays on device
    wblob = np.concatenate(
        [
            np.asarray(w, np.float32).T.astype(bf16)
            for w in (wq, wk, wv, wg, wo)
        ],
        axis=1,
    )
    consts = {
        "wblob": np.ascontiguousarray(wblob),
        # nbT[p, kc*1024 + j*256 + q] = nb[0, h, q, kc*128+p] with the
        # bank-major head order j: [h0, h2, h1, h3] (matches et layout)
        "nbT": np.ascontiguousarray(
            nb[0]
            .transpose(2, 0, 1)  # [k, h, q]
            .reshape(2, 128, H, N)
            .transpose(1, 0, 2, 3)
            .reshape(128, 2 * H * N)
            .astype(bf16)
        ),
    }
    if WITH_BO:
        consts["bo_row"] = np.ascontiguousarray(
            np.asarray(bo, np.float32)[None, :].astype(bf16)
        )
    bgn_col = (-np.asarray(bg, np.float32))[:, None]
    # bias_r[p, kc*RPC + r] = bias[0, n0+r, 0, 0, kc*128+p]
    bias_kn = bias[0, :, 0, 0, :].T.reshape(2, 128, N)  # [kc, p, n]
    in_maps = []
    for c in range(NCORES):
        n0 = c * RPC
        rows = slice(n0, n0 + RPC)
        m = dict(consts)
        m["xt"] = np.ascontiguousarray(q_data[0, rows].transpose(0, 2, 1).astype(bf16))
        m["mt"] = np.ascontiguousarray(m_data[0, rows].transpose(0, 2, 1).astype(bf16))
        bias_r = bias_kn[:, :, rows].transpose(1, 0, 2).reshape(128, 2 * RPC)
        m["fblob"] = np.ascontiguousarray(
            np.concatenate([bgn_col, bias_r], axis=1, dtype=np.float32)
        )
        in_maps.append(m)
    return in_maps


def kernel(**inputs) -> np.ndarray:
    global WITH_BO
    want_bo = bool(np.any(np.asarray(inputs["bo"]) != 0))
    if want_bo != WITH_BO or "nc" not in _CACHE:
        WITH_BO = want_bo
        _CACHE.clear()
    nc, _ = _build()
    in_maps = _prep_in_maps(**inputs)
    res = run_bass_kernel_spmd(nc, in_maps, core_ids=list(range(NCORES)))
    out = np.concatenate(
        [
            res.results[c]["out"].astype(np.float32).transpose(0, 2, 1)
            for c in range(NCORES)
        ],
        axis=0,
    )
    return np.ascontiguousarray(out.reshape(B, N, N, C).astype(np.float32))


if __name__ == "__main__":
    # smoke test against a tiny numpy reference
    rng = np.random.default_rng(0)
    inputs = {
        "q_data": rng.standard_normal((B, N, N, C)).astype(np.float32),
        "m_data": rng.standard_normal((B, N, N, C)).astype(np.float32),
        "bias": rng.standard_normal((B, N, 1, 1, N)).astype(np.float32),
        "nonbatched_bias": rng.standard_normal((1, H, N, N)).astype(np.float32),
        "wq": (rng.standard_normal((C, C)) / np.sqrt(C)).astype(np.float32),
        "wk": (rng.standard_normal((C, C)) / np.sqrt(C)).astype(np.float32),
        "wv": (rng.standard_normal((C, C)) / np.sqrt(C)).astype(np.float32),
        "wo": (rng.standard_normal((C, C)) / np.sqrt(C)).astype(np.float32),
        "bo": np.zeros((C,), np.float32),
        "wg": np.ones((C, C), np.float32) / np.sqrt(C),
        "bg": np.ones((C,), np.float32),
    }
    out = kernel(**inputs)
    print("out", out.shape, out.dtype, float(np.abs(out).max()))
